# revision 3
# baseline (speedup 1.0000x reference)
"""Trainium2 Bass kernel for FBPINN-with-window (dense MoE over 16 subnets).

Math (per point n):
    h   = relu(x @ pW0 + pb0); h += relu(h @ pWmid_l + pbmid_l) (x2)
    z   = h @ pWl + pbl;  ez = exp(z)            (softmax un-normalized)
    xn_c = (x - center_c)/scale_c  (folded on host into layer-0 weights)
    g_c = tanh(xn_c @ W0_c + b0_c); g_c = tanh(g_c @ Wmid_cl + bmid_cl) (x2)
    u_c = g_c @ Wl_c + bl_c
    acc = sum_c softmax(z)_c * u_c = (sum_c ez_c*(g_c@Wl_c) + sum_c ez_c*bl_c) / sum_c ez_c
    out = acc * x0(1-x0)*x1(1-x1)

Device layout: activations transposed (features on partitions, points on the
free dim).  Data-parallel over 8 cores (8192 points each), 4 point-tiles of
2048 per core.  Matmuls run as float32r (full fp32 storage, reduced-precision
multiply at full PE speed).  Mid-layer weights are streamed from HBM per
(tile, subnet) — they don't fit in SBUF alongside the activations.
"""

import os

import numpy as np

N = 65536
D = 2
C = 16
PH = 128
PNMID = 2
SW = 256
SNMID = 2

NCORES = 8
NP = N // NCORES          # 8192 points per core
PTILE = 2048              # points per tile
NT = NP // PTILE          # 4 tiles
CHK = 512                 # matmul moving free dim (one PSUM bank)
NCH = PTILE // CHK        # 4 chunks per tile
FC = SW // 128            # 2 feature chunks
KC = SW // 128            # 2 contraction chunks
PPB = NP // 128           # 64 points per partition (points-layout)
WBLK = SNMID * FC * KC * 128  # mid-weight cols per subnet (1024)

_CACHE = {}


def _build():
    import concourse.mybir as mybir
    import concourse.tile as tile
    from concourse import bacc

    f32 = mybir.dt.float32
    f32r = mybir.dt.float32r
    AF = mybir.ActivationFunctionType
    OP = mybir.AluOpType

    nc = bacc.Bacc("TRN2", debug=False)

    def din(name, shape, dt=f32):
        return nc.dram_tensor(name, shape, dt, kind="ExternalInput").ap()

    xT = din("xT", (D, NP), f32r)
    xP = din("xP", (128, 2 * PPB))
    w0e = din("w0e", (D, C * SW), f32r)
    b0e = din("b0e", (128, C * FC))
    wm = din("wm", (128, C * WBLK), f32r)
    bm = din("bm", (128, C * SNMID * FC))
    wl = din("wl", (128, C * KC), f32r)
    pw0 = din("pw0", (D, PH), f32r)
    pb0 = din("pb0", (PH, 1))
    pwm = din("pwm", (PH, PNMID * PH), f32r)
    pbm = din("pbm", (PH, PNMID))
    pwl = din("pwl", (PH, C), f32r)
    pbl = din("pbl", (C, 1))
    cw = din("cw", (C, 2), f32r)
    y = nc.dram_tensor("y", (NP,), f32, kind="ExternalOutput").ap()

    with tile.TileContext(nc) as tc:
        with (
            tc.tile_pool(name="wp", bufs=1) as wp,
            tc.tile_pool(name="wmp", bufs=3) as wmp,
            tc.tile_pool(name="gp", bufs=3) as gp,
            tc.tile_pool(name="hp", bufs=2) as hp,
            tc.tile_pool(name="sp", bufs=2) as sp,
            tc.tile_pool(name="rp", bufs=3) as rp,
            tc.tile_pool(name="xp", bufs=2) as xpl,
            tc.tile_pool(name="fin", bufs=1) as fin,
            tc.tile_pool(name="pp", bufs=2, space="PSUM") as pp,
        ):
            # ---- small weights / constants into SBUF (resident) ----
            s_pw0 = wp.tile([D, PH], f32r)
            nc.sync.dma_start(s_pw0[:], pw0)
            s_pb0 = wp.tile([PH, 1], f32)
            nc.sync.dma_start(s_pb0[:], pb0)
            s_pwm = wp.tile([PH, PNMID * PH], f32r)
            nc.sync.dma_start(s_pwm[:], pwm)
            s_pbm = wp.tile([PH, PNMID], f32)
            nc.sync.dma_start(s_pbm[:], pbm)
            s_pwl = wp.tile([PH, C], f32r)
            nc.sync.dma_start(s_pwl[:], pwl)
            s_pbl = wp.tile([C, 1], f32)
            nc.sync.dma_start(s_pbl[:], pbl)
            s_cw = wp.tile([C, 2], f32r)
            nc.sync.dma_start(s_cw[:], cw)
            s_w0e = wp.tile([D, C * SW], f32r)
            nc.sync.dma_start(s_w0e[:], w0e)
            s_b0e = wp.tile([128, C * FC], f32)
            nc.sync.dma_start(s_b0e[:], b0e)
            s_wl = wp.tile([128, C * KC], f32r)
            nc.sync.dma_start(s_wl[:], wl)
            s_bm = wp.tile([128, C * SNMID * FC], f32)
            nc.sync.dma_start(s_bm[:], bm)

            # ---- per-core x (points-layout) + boundary factor ----
            s_xP = fin.tile([128, 2 * PPB], f32)
            nc.sync.dma_start(s_xP[:], xP)
            s_xmx = fin.tile([128, 2 * PPB], f32)
            nc.vector.tensor_mul(s_xmx[:], s_xP[:], s_xP[:])
            nc.vector.tensor_sub(s_xmx[:], s_xP[:], s_xmx[:])
            v = s_xmx.rearrange("p (j two) -> p j two", two=2)
            s_bc = fin.tile([128, PPB], f32)
            nc.vector.tensor_mul(s_bc[:], v[:, :, 0], v[:, :, 1])

            # points-layout accumulators, filled per tile via reshape DMAs
            s_accP = fin.tile([128, PPB], f32)
            s_s1P = fin.tile([128, PPB], f32)
            s_s2P = fin.tile([128, PPB], f32)

            for t in range(NT):
                toff = t * PTILE
                s_xt = xpl.tile([D, PTILE], f32r, tag="xt")
                nc.sync.dma_start(s_xt[:], xT[:, toff : toff + PTILE])

                def xchunk(n):
                    return s_xt[:, n * CHK : (n + 1) * CHK]

                # ---------- PoU gating net ----------
                ps0 = pp.tile([PH, PTILE], f32, tag="mm")
                for n in range(NCH):
                    nc.tensor.matmul(
                        ps0[:, n * CHK : (n + 1) * CHK],
                        s_pw0[:],
                        xchunk(n),
                        start=True,
                        stop=True,
                    )
                h = hp.tile([PH, PTILE], f32r, tag="h")
                nc.vector.tensor_scalar(
                    h[:], ps0[:], s_pb0[:, 0:1], 0.0, op0=OP.add, op1=OP.max
                )
                for l in range(PNMID):
                    psl = pp.tile([PH, PTILE], f32, tag="mm")
                    for n in range(NCH):
                        nc.tensor.matmul(
                            psl[:, n * CHK : (n + 1) * CHK],
                            s_pwm[:, l * PH : (l + 1) * PH],
                            h[:, n * CHK : (n + 1) * CHK],
                            start=True,
                            stop=True,
                        )
                    hr = hp.tile([PH, PTILE], f32r, tag="h")
                    nc.vector.tensor_scalar(
                        hr[:], psl[:], s_pbm[:, l : l + 1], 0.0, op0=OP.add, op1=OP.max
                    )
                    nc.vector.tensor_add(hr[:], hr[:], h[:])
                    h = hr
                # logits -> ez
                psz = pp.tile([C, PTILE], f32, tag="mm")
                for n in range(NCH):
                    nc.tensor.matmul(
                        psz[:, n * CHK : (n + 1) * CHK],
                        s_pwl[:],
                        h[:, n * CHK : (n + 1) * CHK],
                        start=True,
                        stop=True,
                    )
                ez = sp.tile([C, PTILE], f32r, tag="ez")
                nc.scalar.activation(ez[:], psz[:], AF.Exp, bias=s_pbl[:, 0:1])
                # s1 = sum_c ez ; s2 = sum_c ez*bl_c
                pss = pp.tile([2, PTILE], f32, tag="mm")
                for n in range(NCH):
                    nc.tensor.matmul(
                        pss[:, n * CHK : (n + 1) * CHK],
                        s_cw[:],
                        ez[:, n * CHK : (n + 1) * CHK],
                        start=True,
                        stop=True,
                    )
                s12row = rp.tile([2, PTILE], f32, tag="row")
                nc.vector.tensor_copy(s12row[:], pss[:])
                nc.sync.dma_start(
                    s_s1P[t * 32 : (t + 1) * 32, :], s12row[0:1, :]
                )
                nc.sync.dma_start(
                    s_s2P[t * 32 : (t + 1) * 32, :], s12row[1:2, :]
                )

                # ---------- subnets ----------
                u_asm = sp.tile([C, PTILE], f32r, tag="ua")
                for c in range(C):
                    # mid-layer weights for this subnet stream from HBM
                    s_wmc = wmp.tile([128, WBLK], f32r, tag="wm")
                    nc.sync.dma_start(
                        s_wmc[:, : WBLK // 2], wm[:, c * WBLK : c * WBLK + WBLK // 2]
                    )
                    nc.sync.dma_start(
                        s_wmc[:, WBLK // 2 :],
                        wm[:, c * WBLK + WBLK // 2 : (c + 1) * WBLK],
                    )
                    # layer 0 (K=2)
                    g0 = gp.tile([128, KC, PTILE], f32r, tag="g")
                    for fc in range(FC):
                        pt = pp.tile([128, PTILE], f32, tag="mm")
                        for n in range(NCH):
                            nc.tensor.matmul(
                                pt[:, n * CHK : (n + 1) * CHK],
                                s_w0e[:, c * SW + fc * 128 : c * SW + fc * 128 + 128],
                                xchunk(n),
                                start=True,
                                stop=True,
                            )
                        nc.scalar.activation(
                            g0[:, fc, :],
                            pt[:],
                            AF.Tanh,
                            bias=s_b0e[:, c * FC + fc : c * FC + fc + 1],
                        )
                    gcur = g0
                    # mid layers (K=256)
                    for l in range(SNMID):
                        gn = gp.tile([128, KC, PTILE], f32r, tag="g")
                        for fc in range(FC):
                            pt = pp.tile([128, PTILE], f32, tag="mm")
                            for kc in range(KC):
                                col = ((l * FC + fc) * KC + kc) * 128
                                for n in range(NCH):
                                    nc.tensor.matmul(
                                        pt[:, n * CHK : (n + 1) * CHK],
                                        s_wmc[:, col : col + 128],
                                        gcur[:, kc, n * CHK : (n + 1) * CHK],
                                        start=(kc == 0),
                                        stop=(kc == KC - 1),
                                    )
                            bcol = (c * SNMID + l) * FC + fc
                            nc.scalar.activation(
                                gn[:, fc, :],
                                pt[:],
                                AF.Tanh,
                                bias=s_bm[:, bcol : bcol + 1],
                            )
                        gcur = gn
                    # last layer (M=1) -> u_c row
                    pu = pp.tile([1, PTILE], f32, tag="mm")
                    for kc in range(KC):
                        wcol = c * KC + kc
                        for n in range(NCH):
                            nc.tensor.matmul(
                                pu[:, n * CHK : (n + 1) * CHK],
                                s_wl[:, wcol : wcol + 1],
                                gcur[:, kc, n * CHK : (n + 1) * CHK],
                                start=(kc == 0),
                                stop=(kc == KC - 1),
                            )
                    urow = rp.tile([2, PTILE], f32r, tag="row")
                    nc.vector.tensor_copy(urow[0:1, :], pu[:])
                    nc.sync.dma_start(u_asm[c : c + 1, :], urow[0:1, :])

                # ---------- windowed combine ----------
                nc.vector.tensor_mul(u_asm[:], u_asm[:], ez[:])
                pacc = pp.tile([1, PTILE], f32, tag="mm")
                for n in range(NCH):
                    nc.tensor.matmul(
                        pacc[:, n * CHK : (n + 1) * CHK],
                        s_cw[:, 0:1],
                        u_asm[:, n * CHK : (n + 1) * CHK],
                        start=True,
                        stop=True,
                    )
                accrow = rp.tile([2, PTILE], f32, tag="row")
                nc.vector.tensor_copy(accrow[0:1, :], pacc[:])
                nc.sync.dma_start(
                    s_accP[t * 32 : (t + 1) * 32, :], accrow[0:1, :]
                )

            # ---------- final: combine in points-layout ----------
            s_r = fin.tile([128, PPB], f32)
            nc.vector.reciprocal(s_r[:], s_s1P[:])
            s_num = fin.tile([128, PPB], f32)
            nc.vector.tensor_add(s_num[:], s_accP[:], s_s2P[:])
            nc.vector.tensor_mul(s_num[:], s_num[:], s_r[:])
            nc.vector.tensor_mul(s_num[:], s_num[:], s_bc[:])
            nc.sync.dma_start(y.rearrange("(p j) -> p j", p=128), s_num[:])

    nc.compile()
    return nc


def _prep_inputs(inputs):
    f = lambda k: np.ascontiguousarray(np.asarray(inputs[k]), dtype=np.float32)
    x = f("x")
    centers, scales = f("centers"), f("scales")
    sub_W0, sub_b0 = f("sub_W0"), f("sub_b0")
    sub_Wmid, sub_bmid = f("sub_Wmid"), f("sub_bmid")
    sub_Wl, sub_bl = f("sub_Wl"), f("sub_bl")

    # fold per-subdomain normalization into layer-0 weights:
    # xn = (x - c)/s  =>  xn @ W0 + b0 = x @ (W0/s) + (b0 - (c/s) @ W0)
    w0e_full = sub_W0 / scales[:, :, None]                       # [C, D, SW]
    b0e_full = sub_b0 - np.einsum("cd,cdw->cw", centers / scales, sub_W0)

    w0e = np.ascontiguousarray(w0e_full.transpose(1, 0, 2).reshape(D, C * SW))
    b0e = np.ascontiguousarray(
        b0e_full.reshape(C, FC, 128).transpose(2, 0, 1).reshape(128, C * FC)
    )
    wm = np.ascontiguousarray(
        sub_Wmid.reshape(C, SNMID, KC, 128, FC, 128)
        .transpose(3, 0, 1, 4, 2, 5)
        .reshape(128, C * WBLK)
    )
    bm = np.ascontiguousarray(
        sub_bmid.reshape(C, SNMID, FC, 128).transpose(3, 0, 1, 2).reshape(128, -1)
    )
    wl = np.ascontiguousarray(
        sub_Wl.reshape(C, KC, 128).transpose(2, 0, 1).reshape(128, -1)
    )
    cwm = np.ascontiguousarray(
        np.stack([np.ones(C, np.float32), sub_bl[:, 0]], axis=1)
    )

    shared = dict(
        w0e=w0e,
        b0e=b0e,
        wm=wm,
        bm=bm,
        wl=wl,
        pw0=f("pou_W0"),
        pb0=np.ascontiguousarray(f("pou_b0")[:, None]),
        pwm=np.ascontiguousarray(f("pou_Wmid").transpose(1, 0, 2).reshape(PH, -1)),
        pbm=np.ascontiguousarray(f("pou_bmid").T),
        pwl=f("pou_Wl"),
        pbl=np.ascontiguousarray(f("pou_bl")[:, None]),
        cw=cwm,
    )

    in_maps = []
    for core in range(NCORES):
        xs = x[core * NP : (core + 1) * NP]
        m = dict(shared)
        m["xT"] = np.ascontiguousarray(xs.T)
        m["xP"] = np.ascontiguousarray(xs.reshape(128, 2 * PPB))
        in_maps.append(m)
    return in_maps


def kernel(**inputs):
    from concourse.bass_utils import run_bass_kernel_spmd

    if "nc" not in _CACHE:
        _CACHE["nc"] = _build()
    nc = _CACHE["nc"]

    in_maps = _prep_inputs(inputs)
    trace = os.environ.get("KERNEL_TRACE", "0") == "1"
    res = run_bass_kernel_spmd(
        nc, in_maps, core_ids=list(range(NCORES)), trace=trace
    )
    kernel.last_results = res
    y = np.concatenate([res.results[i]["y"] for i in range(NCORES)])
    return y.astype(np.float32)


# revision 4
# speedup vs baseline: 1.0434x; 1.0434x over previous
"""Trainium2 Bass kernel for FBPINN-with-window (dense MoE over 16 subnets).

Math (per point n):
    h   = relu(x @ pW0 + pb0); h += relu(h @ pWmid_l + pbmid_l) (x2)
    z   = h @ pWl + pbl;  ez = exp(z)            (softmax un-normalized)
    xn_c = (x - center_c)/scale_c  (folded on host into layer-0 weights)
    g_c = tanh(xn_c @ W0_c + b0_c); g_c = tanh(g_c @ Wmid_cl + bmid_cl) (x2)
    u_c = g_c @ Wl_c + bl_c
    acc = sum_c softmax(z)_c * u_c = (sum_c ez_c*(g_c@Wl_c) + sum_c ez_c*bl_c) / sum_c ez_c
    out = acc * x0(1-x0)*x1(1-x1)

Device layout: activations transposed (features on partitions, points on the
free dim).  Data-parallel over 8 cores (8192 points each), 4 point-tiles of
2048 per core.  Matmuls run as float32r (full fp32 storage, reduced-precision
multiply at full PE speed).  Mid-layer weights are streamed from HBM per
(tile, subnet) — they don't fit in SBUF alongside the activations.
"""

import os

import numpy as np

N = 65536
D = 2
C = 16
PH = 128
PNMID = 2
SW = 256
SNMID = 2

NCORES = 8
NP = N // NCORES          # 8192 points per core
PTILE = 2048              # points per tile
NT = NP // PTILE          # 4 tiles
CHK = 512                 # matmul moving free dim (one PSUM bank)
NCH = PTILE // CHK        # 4 chunks per tile
FC = SW // 128            # 2 feature chunks
KC = SW // 128            # 2 contraction chunks
PPB = NP // 128           # 64 points per partition (points-layout)
WBLK = SNMID * FC * KC * 128  # mid-weight cols per subnet (1024)

_CACHE = {}


def _build():
    import concourse.mybir as mybir
    import concourse.tile as tile
    from concourse import bacc

    f32 = mybir.dt.float32
    f32r = mybir.dt.float32r
    bf16 = mybir.dt.bfloat16
    AF = mybir.ActivationFunctionType
    OP = mybir.AluOpType

    nc = bacc.Bacc("TRN2", debug=False)

    def din(name, shape, dt=f32):
        return nc.dram_tensor(name, shape, dt, kind="ExternalInput").ap()

    xT = din("xT", (D, NP), f32r)
    xP = din("xP", (128, 2 * PPB))
    w0e = din("w0e", (D, C * SW), f32r)
    b0e = din("b0e", (128, C * FC))
    wm = din("wm", (128, C * WBLK), bf16)
    bm = din("bm", (128, C * SNMID * FC))
    wl = din("wl", (128, C * KC), bf16)
    pw0 = din("pw0", (D, PH), f32r)
    pb0 = din("pb0", (PH, 1))
    pwm = din("pwm", (PH, PNMID * PH), f32r)
    pbm = din("pbm", (PH, PNMID))
    pwl = din("pwl", (PH, C), f32r)
    pbl = din("pbl", (C, 1))
    cw = din("cw", (C, 2), f32r)
    y = nc.dram_tensor("y", (NP,), f32, kind="ExternalOutput").ap()

    with tile.TileContext(nc) as tc:
        with (
            tc.tile_pool(name="wp", bufs=1) as wp,
            tc.tile_pool(name="wmp", bufs=3) as wmp,
            tc.tile_pool(name="gp", bufs=3) as gp,
            tc.tile_pool(name="hp", bufs=2) as hp,
            tc.tile_pool(name="sp", bufs=2) as sp,
            tc.tile_pool(name="rp", bufs=3) as rp,
            tc.tile_pool(name="xp", bufs=2) as xpl,
            tc.tile_pool(name="fin", bufs=1) as fin,
            tc.tile_pool(name="pp", bufs=2, space="PSUM") as pp,
        ):
            # ---- small weights / constants into SBUF (resident) ----
            s_pw0 = wp.tile([D, PH], f32r)
            nc.sync.dma_start(s_pw0[:], pw0)
            s_pb0 = wp.tile([PH, 1], f32)
            nc.sync.dma_start(s_pb0[:], pb0)
            s_pwm = wp.tile([PH, PNMID * PH], f32r)
            nc.sync.dma_start(s_pwm[:], pwm)
            s_pbm = wp.tile([PH, PNMID], f32)
            nc.sync.dma_start(s_pbm[:], pbm)
            s_pwl = wp.tile([PH, C], f32r)
            nc.sync.dma_start(s_pwl[:], pwl)
            s_pbl = wp.tile([C, 1], f32)
            nc.sync.dma_start(s_pbl[:], pbl)
            s_cw = wp.tile([C, 2], f32r)
            nc.sync.dma_start(s_cw[:], cw)
            s_w0e = wp.tile([D, C * SW], f32r)
            nc.sync.dma_start(s_w0e[:], w0e)
            s_b0e = wp.tile([128, C * FC], f32)
            nc.sync.dma_start(s_b0e[:], b0e)
            s_wl = wp.tile([128, C * KC], bf16)
            nc.sync.dma_start(s_wl[:], wl)
            s_bm = wp.tile([128, C * SNMID * FC], f32)
            nc.sync.dma_start(s_bm[:], bm)

            # ---- per-core x (points-layout) + boundary factor ----
            s_xP = fin.tile([128, 2 * PPB], f32)
            nc.sync.dma_start(s_xP[:], xP)
            s_xmx = fin.tile([128, 2 * PPB], f32)
            nc.vector.tensor_mul(s_xmx[:], s_xP[:], s_xP[:])
            nc.vector.tensor_sub(s_xmx[:], s_xP[:], s_xmx[:])
            v = s_xmx.rearrange("p (j two) -> p j two", two=2)
            s_bc = fin.tile([128, PPB], f32)
            nc.vector.tensor_mul(s_bc[:], v[:, :, 0], v[:, :, 1])

            # points-layout accumulators, filled per tile via reshape DMAs
            s_accP = fin.tile([128, PPB], f32)
            s_s1P = fin.tile([128, PPB], f32)
            s_s2P = fin.tile([128, PPB], f32)

            for t in range(NT):
                toff = t * PTILE
                s_xt = xpl.tile([D, PTILE], f32r, tag="xt")
                nc.sync.dma_start(s_xt[:], xT[:, toff : toff + PTILE])

                def xchunk(n):
                    return s_xt[:, n * CHK : (n + 1) * CHK]

                # ---------- PoU gating net ----------
                ps0 = pp.tile([PH, PTILE], f32, tag="mm")
                for n in range(NCH):
                    nc.tensor.matmul(
                        ps0[:, n * CHK : (n + 1) * CHK],
                        s_pw0[:],
                        xchunk(n),
                        start=True,
                        stop=True,
                    )
                h = hp.tile([PH, PTILE], f32r, tag="h")
                nc.vector.tensor_scalar(
                    h[:], ps0[:], s_pb0[:, 0:1], 0.0, op0=OP.add, op1=OP.max
                )
                for l in range(PNMID):
                    psl = pp.tile([PH, PTILE], f32, tag="mm")
                    for n in range(NCH):
                        nc.tensor.matmul(
                            psl[:, n * CHK : (n + 1) * CHK],
                            s_pwm[:, l * PH : (l + 1) * PH],
                            h[:, n * CHK : (n + 1) * CHK],
                            start=True,
                            stop=True,
                        )
                    hr = hp.tile([PH, PTILE], f32r, tag="h")
                    nc.vector.tensor_scalar(
                        hr[:], psl[:], s_pbm[:, l : l + 1], 0.0, op0=OP.add, op1=OP.max
                    )
                    nc.vector.tensor_add(hr[:], hr[:], h[:])
                    h = hr
                # logits -> ez
                psz = pp.tile([C, PTILE], f32, tag="mm")
                for n in range(NCH):
                    nc.tensor.matmul(
                        psz[:, n * CHK : (n + 1) * CHK],
                        s_pwl[:],
                        h[:, n * CHK : (n + 1) * CHK],
                        start=True,
                        stop=True,
                    )
                ez = sp.tile([C, PTILE], f32r, tag="ez")
                nc.scalar.activation(ez[:], psz[:], AF.Exp, bias=s_pbl[:, 0:1])
                # s1 = sum_c ez ; s2 = sum_c ez*bl_c
                pss = pp.tile([2, PTILE], f32, tag="mm")
                for n in range(NCH):
                    nc.tensor.matmul(
                        pss[:, n * CHK : (n + 1) * CHK],
                        s_cw[:],
                        ez[:, n * CHK : (n + 1) * CHK],
                        start=True,
                        stop=True,
                    )
                s12row = rp.tile([2, PTILE], f32, tag="row")
                nc.vector.tensor_copy(s12row[:], pss[:])
                nc.sync.dma_start(
                    s_s1P[t * 32 : (t + 1) * 32, :], s12row[0:1, :]
                )
                nc.sync.dma_start(
                    s_s2P[t * 32 : (t + 1) * 32, :], s12row[1:2, :]
                )

                # ---------- subnets ----------
                u_asm = sp.tile([C, PTILE], f32r, tag="ua")
                for c in range(C):
                    # mid-layer weights for this subnet stream from HBM
                    s_wmc = wmp.tile([128, WBLK], bf16, tag="wm")
                    nc.sync.dma_start(
                        s_wmc[:, : WBLK // 2], wm[:, c * WBLK : c * WBLK + WBLK // 2]
                    )
                    nc.sync.dma_start(
                        s_wmc[:, WBLK // 2 :],
                        wm[:, c * WBLK + WBLK // 2 : (c + 1) * WBLK],
                    )
                    # layer 0 (K=2)
                    g0 = gp.tile([128, KC, PTILE], bf16, tag="g")
                    for fc in range(FC):
                        pt = pp.tile([128, PTILE], f32, tag="mm")
                        for n in range(NCH):
                            nc.tensor.matmul(
                                pt[:, n * CHK : (n + 1) * CHK],
                                s_w0e[:, c * SW + fc * 128 : c * SW + fc * 128 + 128],
                                xchunk(n),
                                start=True,
                                stop=True,
                            )
                        nc.scalar.activation(
                            g0[:, fc, :],
                            pt[:],
                            AF.Tanh,
                            bias=s_b0e[:, c * FC + fc : c * FC + fc + 1],
                        )
                    gcur = g0
                    # mid layers (K=256)
                    for l in range(SNMID):
                        gn = gp.tile([128, KC, PTILE], bf16, tag="g")
                        for fc in range(FC):
                            pt = pp.tile([128, PTILE], f32, tag="mm")
                            for kc in range(KC):
                                col = ((l * FC + fc) * KC + kc) * 128
                                for n in range(NCH):
                                    nc.tensor.matmul(
                                        pt[:, n * CHK : (n + 1) * CHK],
                                        s_wmc[:, col : col + 128],
                                        gcur[:, kc, n * CHK : (n + 1) * CHK],
                                        start=(kc == 0),
                                        stop=(kc == KC - 1),
                                    )
                            bcol = (c * SNMID + l) * FC + fc
                            nc.scalar.activation(
                                gn[:, fc, :],
                                pt[:],
                                AF.Tanh,
                                bias=s_bm[:, bcol : bcol + 1],
                            )
                        gcur = gn
                    # last layer (M=1) -> u_c row
                    pu = pp.tile([1, PTILE], f32, tag="mm")
                    for kc in range(KC):
                        wcol = c * KC + kc
                        for n in range(NCH):
                            nc.tensor.matmul(
                                pu[:, n * CHK : (n + 1) * CHK],
                                s_wl[:, wcol : wcol + 1],
                                gcur[:, kc, n * CHK : (n + 1) * CHK],
                                start=(kc == 0),
                                stop=(kc == KC - 1),
                            )
                    urow = rp.tile([2, PTILE], f32r, tag="row")
                    nc.vector.tensor_copy(urow[0:1, :], pu[:])
                    nc.sync.dma_start(u_asm[c : c + 1, :], urow[0:1, :])

                # ---------- windowed combine ----------
                nc.vector.tensor_mul(u_asm[:], u_asm[:], ez[:])
                pacc = pp.tile([1, PTILE], f32, tag="mm")
                for n in range(NCH):
                    nc.tensor.matmul(
                        pacc[:, n * CHK : (n + 1) * CHK],
                        s_cw[:, 0:1],
                        u_asm[:, n * CHK : (n + 1) * CHK],
                        start=True,
                        stop=True,
                    )
                accrow = rp.tile([2, PTILE], f32, tag="row")
                nc.vector.tensor_copy(accrow[0:1, :], pacc[:])
                nc.sync.dma_start(
                    s_accP[t * 32 : (t + 1) * 32, :], accrow[0:1, :]
                )

            # ---------- final: combine in points-layout ----------
            s_r = fin.tile([128, PPB], f32)
            nc.vector.reciprocal(s_r[:], s_s1P[:])
            s_num = fin.tile([128, PPB], f32)
            nc.vector.tensor_add(s_num[:], s_accP[:], s_s2P[:])
            nc.vector.tensor_mul(s_num[:], s_num[:], s_r[:])
            nc.vector.tensor_mul(s_num[:], s_num[:], s_bc[:])
            nc.sync.dma_start(y.rearrange("(p j) -> p j", p=128), s_num[:])

    nc.compile()
    return nc


def _prep_inputs(inputs):
    f = lambda k: np.ascontiguousarray(np.asarray(inputs[k]), dtype=np.float32)
    x = f("x")
    centers, scales = f("centers"), f("scales")
    sub_W0, sub_b0 = f("sub_W0"), f("sub_b0")
    sub_Wmid, sub_bmid = f("sub_Wmid"), f("sub_bmid")
    sub_Wl, sub_bl = f("sub_Wl"), f("sub_bl")

    # fold per-subdomain normalization into layer-0 weights:
    # xn = (x - c)/s  =>  xn @ W0 + b0 = x @ (W0/s) + (b0 - (c/s) @ W0)
    w0e_full = sub_W0 / scales[:, :, None]                       # [C, D, SW]
    b0e_full = sub_b0 - np.einsum("cd,cdw->cw", centers / scales, sub_W0)

    w0e = np.ascontiguousarray(w0e_full.transpose(1, 0, 2).reshape(D, C * SW))
    b0e = np.ascontiguousarray(
        b0e_full.reshape(C, FC, 128).transpose(2, 0, 1).reshape(128, C * FC)
    )
    import ml_dtypes

    wm = np.ascontiguousarray(
        sub_Wmid.reshape(C, SNMID, KC, 128, FC, 128)
        .transpose(3, 0, 1, 4, 2, 5)
        .reshape(128, C * WBLK)
    ).astype(ml_dtypes.bfloat16)
    bm = np.ascontiguousarray(
        sub_bmid.reshape(C, SNMID, FC, 128).transpose(3, 0, 1, 2).reshape(128, -1)
    )
    wl = np.ascontiguousarray(
        sub_Wl.reshape(C, KC, 128).transpose(2, 0, 1).reshape(128, -1)
    ).astype(ml_dtypes.bfloat16)
    cwm = np.ascontiguousarray(
        np.stack([np.ones(C, np.float32), sub_bl[:, 0]], axis=1)
    )

    shared = dict(
        w0e=w0e,
        b0e=b0e,
        wm=wm,
        bm=bm,
        wl=wl,
        pw0=f("pou_W0"),
        pb0=np.ascontiguousarray(f("pou_b0")[:, None]),
        pwm=np.ascontiguousarray(f("pou_Wmid").transpose(1, 0, 2).reshape(PH, -1)),
        pbm=np.ascontiguousarray(f("pou_bmid").T),
        pwl=f("pou_Wl"),
        pbl=np.ascontiguousarray(f("pou_bl")[:, None]),
        cw=cwm,
    )

    in_maps = []
    for core in range(NCORES):
        xs = x[core * NP : (core + 1) * NP]
        m = dict(shared)
        m["xT"] = np.ascontiguousarray(xs.T)
        m["xP"] = np.ascontiguousarray(xs.reshape(128, 2 * PPB))
        in_maps.append(m)
    return in_maps


def kernel(**inputs):
    from concourse.bass_utils import run_bass_kernel_spmd

    if "nc" not in _CACHE:
        _CACHE["nc"] = _build()
    nc = _CACHE["nc"]

    in_maps = _prep_inputs(inputs)
    trace = os.environ.get("KERNEL_TRACE", "0") == "1"
    res = run_bass_kernel_spmd(
        nc, in_maps, core_ids=list(range(NCORES)), trace=trace
    )
    kernel.last_results = res
    y = np.concatenate([res.results[i]["y"] for i in range(NCORES)])
    return y.astype(np.float32)


# revision 6
# speedup vs baseline: 1.0468x; 1.0033x over previous
"""Trainium2 Bass kernel for FBPINN-with-window (dense MoE over 16 subnets).

Math (per point n):
    h   = relu(x @ pW0 + pb0); h += relu(h @ pWmid_l + pbmid_l) (x2)
    z   = h @ pWl + pbl;  ez = exp(z)            (softmax un-normalized)
    xn_c = (x - center_c)/scale_c  (folded on host into layer-0 weights)
    g_c = tanh(xn_c @ W0_c + b0_c); g_c = tanh(g_c @ Wmid_cl + bmid_cl) (x2)
    u_c = g_c @ Wl_c + bl_c
    acc = sum_c softmax(z)_c * u_c = (sum_c ez_c*(g_c@Wl_c) + sum_c ez_c*bl_c) / sum_c ez_c
    out = acc * x0(1-x0)*x1(1-x1)

Device layout: activations transposed (features on partitions, points on the
free dim).  Data-parallel over 8 cores (8192 points each), 4 point-tiles of
2048 per core.  Matmuls run as float32r (full fp32 storage, reduced-precision
multiply at full PE speed).  Mid-layer weights are streamed from HBM per
(tile, subnet) — they don't fit in SBUF alongside the activations.
"""

import os

import numpy as np

N = 65536
D = 2
C = 16
PH = 128
PNMID = 2
SW = 256
SNMID = 2

NCORES = 8
NP = N // NCORES          # 8192 points per core
PTILE = 2048              # points per tile
NT = NP // PTILE          # 4 tiles
CHK = 512                 # matmul moving free dim (one PSUM bank)
NCH = PTILE // CHK        # 4 chunks per tile
FC = SW // 128            # 2 feature chunks
KC = SW // 128            # 2 contraction chunks
PPB = NP // 128           # 64 points per partition (points-layout)
WBLK = SNMID * FC * KC * 128  # mid-weight cols per subnet (1024)

MID_BF16 = os.environ.get("KERNEL_BF16", "0") == "1"

_CACHE = {}


def _build():
    import concourse.mybir as mybir
    import concourse.tile as tile
    from concourse import bacc

    f32 = mybir.dt.float32
    f32r = mybir.dt.float32r
    bf16 = mybir.dt.bfloat16
    AF = mybir.ActivationFunctionType
    OP = mybir.AluOpType

    nc = bacc.Bacc("TRN2", debug=False)

    def din(name, shape, dt=f32):
        return nc.dram_tensor(name, shape, dt, kind="ExternalInput").ap()

    xT = din("xT", (D, NP), f32r)
    xP = din("xP", (128, 2 * PPB))
    w0e = din("w0e", (D, C * SW), f32r)
    b0e = din("b0e", (128, C * FC))
    middt = bf16 if MID_BF16 else f32r
    wm = din("wm", (128, C * WBLK), middt)
    bm = din("bm", (128, C * SNMID * FC))
    wl = din("wl", (128, C * KC), middt)
    pw0 = din("pw0", (D, PH), f32r)
    pb0 = din("pb0", (PH, 1))
    pwm = din("pwm", (PH, PNMID * PH), f32r)
    pbm = din("pbm", (PH, PNMID))
    pwl = din("pwl", (PH, C), f32r)
    pbl = din("pbl", (C, 1))
    cw = din("cw", (C, 2), f32r)
    y = nc.dram_tensor("y", (NP,), f32, kind="ExternalOutput").ap()

    with tile.TileContext(nc) as tc:
        with (
            tc.tile_pool(name="wp", bufs=1) as wp,
            tc.tile_pool(name="wmp", bufs=3) as wmp,
            tc.tile_pool(name="gp", bufs=3) as gp,
            tc.tile_pool(name="hp", bufs=2) as hp,
            tc.tile_pool(name="sp", bufs=2) as sp,
            tc.tile_pool(name="rp", bufs=3) as rp,
            tc.tile_pool(name="xp", bufs=2) as xpl,
            tc.tile_pool(name="fin", bufs=1) as fin,
            tc.tile_pool(name="pp", bufs=2, space="PSUM") as pp,
        ):
            # ---- small weights / constants into SBUF (resident) ----
            s_pw0 = wp.tile([D, PH], f32r)
            nc.sync.dma_start(s_pw0[:], pw0)
            s_pb0 = wp.tile([PH, 1], f32)
            nc.sync.dma_start(s_pb0[:], pb0)
            s_pwm = wp.tile([PH, PNMID * PH], f32r)
            nc.sync.dma_start(s_pwm[:], pwm)
            s_pbm = wp.tile([PH, PNMID], f32)
            nc.sync.dma_start(s_pbm[:], pbm)
            s_pwl = wp.tile([PH, C], f32r)
            nc.sync.dma_start(s_pwl[:], pwl)
            s_pbl = wp.tile([C, 1], f32)
            nc.sync.dma_start(s_pbl[:], pbl)
            s_cw = wp.tile([C, 2], f32r)
            nc.sync.dma_start(s_cw[:], cw)
            s_w0e = wp.tile([D, C * SW], f32r)
            nc.sync.dma_start(s_w0e[:], w0e)
            s_b0e = wp.tile([128, C * FC], f32)
            nc.sync.dma_start(s_b0e[:], b0e)
            s_wl = wp.tile([128, C * KC], middt)
            nc.sync.dma_start(s_wl[:], wl)
            s_bm = wp.tile([128, C * SNMID * FC], f32)
            nc.sync.dma_start(s_bm[:], bm)

            # ---- per-core x (points-layout) + boundary factor ----
            s_xP = fin.tile([128, 2 * PPB], f32)
            nc.sync.dma_start(s_xP[:], xP)
            s_xmx = fin.tile([128, 2 * PPB], f32)
            nc.vector.tensor_mul(s_xmx[:], s_xP[:], s_xP[:])
            nc.vector.tensor_sub(s_xmx[:], s_xP[:], s_xmx[:])
            v = s_xmx.rearrange("p (j two) -> p j two", two=2)
            s_bc = fin.tile([128, PPB], f32)
            nc.vector.tensor_mul(s_bc[:], v[:, :, 0], v[:, :, 1])

            # points-layout accumulators, filled per tile via reshape DMAs
            s_accP = fin.tile([128, PPB], f32)
            s_s1P = fin.tile([128, PPB], f32)
            s_s2P = fin.tile([128, PPB], f32)

            for t in range(NT):
                toff = t * PTILE
                s_xt = xpl.tile([D, PTILE], f32r, tag="xt")
                nc.sync.dma_start(s_xt[:], xT[:, toff : toff + PTILE])

                def xchunk(n):
                    return s_xt[:, n * CHK : (n + 1) * CHK]

                # ---------- PoU gating net ----------
                ps0 = pp.tile([PH, PTILE], f32, tag="mm")
                for n in range(NCH):
                    nc.tensor.matmul(
                        ps0[:, n * CHK : (n + 1) * CHK],
                        s_pw0[:],
                        xchunk(n),
                        start=True,
                        stop=True,
                    )
                h = hp.tile([PH, PTILE], f32r, tag="h")
                nc.vector.tensor_scalar(
                    h[:], ps0[:], s_pb0[:, 0:1], 0.0, op0=OP.add, op1=OP.max
                )
                for l in range(PNMID):
                    psl = pp.tile([PH, PTILE], f32, tag="mm")
                    for n in range(NCH):
                        nc.tensor.matmul(
                            psl[:, n * CHK : (n + 1) * CHK],
                            s_pwm[:, l * PH : (l + 1) * PH],
                            h[:, n * CHK : (n + 1) * CHK],
                            start=True,
                            stop=True,
                        )
                    hr = hp.tile([PH, PTILE], f32r, tag="h")
                    nc.vector.tensor_scalar(
                        hr[:], psl[:], s_pbm[:, l : l + 1], 0.0, op0=OP.add, op1=OP.max
                    )
                    nc.vector.tensor_add(hr[:], hr[:], h[:])
                    h = hr
                # logits -> ez
                psz = pp.tile([C, PTILE], f32, tag="mm")
                for n in range(NCH):
                    nc.tensor.matmul(
                        psz[:, n * CHK : (n + 1) * CHK],
                        s_pwl[:],
                        h[:, n * CHK : (n + 1) * CHK],
                        start=True,
                        stop=True,
                    )
                ez = sp.tile([C, PTILE], f32r, tag="ez")
                nc.scalar.activation(ez[:], psz[:], AF.Exp, bias=s_pbl[:, 0:1])
                # s1 = sum_c ez ; s2 = sum_c ez*bl_c
                pss = pp.tile([2, PTILE], f32, tag="mm")
                for n in range(NCH):
                    nc.tensor.matmul(
                        pss[:, n * CHK : (n + 1) * CHK],
                        s_cw[:],
                        ez[:, n * CHK : (n + 1) * CHK],
                        start=True,
                        stop=True,
                    )
                s12row = rp.tile([2, PTILE], f32, tag="row")
                nc.vector.tensor_copy(s12row[:], pss[:])
                nc.sync.dma_start(
                    s_s1P[t * 32 : (t + 1) * 32, :], s12row[0:1, :]
                )
                nc.sync.dma_start(
                    s_s2P[t * 32 : (t + 1) * 32, :], s12row[1:2, :]
                )

                # ---------- subnets (two chains interleaved) ----------
                u_asm = sp.tile([C, PTILE], f32r, tag="ua")

                def sub_dma(c, st):
                    s_wmc = wmp.tile([128, WBLK], middt, tag="wm")
                    nc.sync.dma_start(
                        s_wmc[:, : WBLK // 2],
                        wm[:, c * WBLK : c * WBLK + WBLK // 2],
                    )
                    nc.sync.dma_start(
                        s_wmc[:, WBLK // 2 :],
                        wm[:, c * WBLK + WBLK // 2 : (c + 1) * WBLK],
                    )
                    st["wm"] = s_wmc

                def sub_l0(c, st, fc):
                    if fc == 0:
                        st["g0"] = gp.tile([128, KC, PTILE], middt, tag="g", name="g0")
                    pt = pp.tile([128, PTILE], f32, tag="mm")
                    for n in range(NCH):
                        nc.tensor.matmul(
                            pt[:, n * CHK : (n + 1) * CHK],
                            s_w0e[:, c * SW + fc * 128 : c * SW + fc * 128 + 128],
                            xchunk(n),
                            start=True,
                            stop=True,
                        )
                    nc.scalar.activation(
                        st["g0"][:, fc, :],
                        pt[:],
                        AF.Tanh,
                        bias=s_b0e[:, c * FC + fc : c * FC + fc + 1],
                    )

                def sub_mid(c, st, l, fc):
                    gcur = st[f"g{l}"]
                    if fc == 0:
                        st[f"g{l + 1}"] = gp.tile([128, KC, PTILE], middt, tag="g", name=f"g{l + 1}")
                    pt = pp.tile([128, PTILE], f32, tag="mm")
                    for kc in range(KC):
                        col = ((l * FC + fc) * KC + kc) * 128
                        for n in range(NCH):
                            nc.tensor.matmul(
                                pt[:, n * CHK : (n + 1) * CHK],
                                st["wm"][:, col : col + 128],
                                gcur[:, kc, n * CHK : (n + 1) * CHK],
                                start=(kc == 0),
                                stop=(kc == KC - 1),
                            )
                    bcol = (c * SNMID + l) * FC + fc
                    nc.scalar.activation(
                        st[f"g{l + 1}"][:, fc, :],
                        pt[:],
                        AF.Tanh,
                        bias=s_bm[:, bcol : bcol + 1],
                    )

                def sub_last(c, st):
                    gcur = st[f"g{SNMID}"]
                    pu = pp.tile([1, PTILE], f32, tag="mm")
                    for kc in range(KC):
                        wcol = c * KC + kc
                        for n in range(NCH):
                            nc.tensor.matmul(
                                pu[:, n * CHK : (n + 1) * CHK],
                                s_wl[:, wcol : wcol + 1],
                                gcur[:, kc, n * CHK : (n + 1) * CHK],
                                start=(kc == 0),
                                stop=(kc == KC - 1),
                            )
                    urow = rp.tile([2, PTILE], f32r, tag="row")
                    nc.vector.tensor_copy(urow[0:1, :], pu[:])
                    nc.sync.dma_start(u_asm[c : c + 1, :], urow[0:1, :])

                def sub_step(c, st, k):
                    if k == 0:
                        sub_dma(c, st)
                    elif k <= 2:
                        sub_l0(c, st, k - 1)
                    elif k <= 6:
                        sub_mid(c, st, (k - 3) // FC, (k - 3) % FC)
                    else:
                        sub_last(c, st)

                for pair in range(C // 2):
                    ca, cb = 2 * pair, 2 * pair + 1
                    sa, sb = {}, {}
                    for k in range(8):
                        sub_step(ca, sa, k)
                        sub_step(cb, sb, k)

                # ---------- windowed combine ----------
                nc.vector.tensor_mul(u_asm[:], u_asm[:], ez[:])
                pacc = pp.tile([1, PTILE], f32, tag="mm")
                for n in range(NCH):
                    nc.tensor.matmul(
                        pacc[:, n * CHK : (n + 1) * CHK],
                        s_cw[:, 0:1],
                        u_asm[:, n * CHK : (n + 1) * CHK],
                        start=True,
                        stop=True,
                    )
                accrow = rp.tile([2, PTILE], f32, tag="row")
                nc.vector.tensor_copy(accrow[0:1, :], pacc[:])
                nc.sync.dma_start(
                    s_accP[t * 32 : (t + 1) * 32, :], accrow[0:1, :]
                )

            # ---------- final: combine in points-layout ----------
            s_r = fin.tile([128, PPB], f32)
            nc.vector.reciprocal(s_r[:], s_s1P[:])
            s_num = fin.tile([128, PPB], f32)
            nc.vector.tensor_add(s_num[:], s_accP[:], s_s2P[:])
            nc.vector.tensor_mul(s_num[:], s_num[:], s_r[:])
            nc.vector.tensor_mul(s_num[:], s_num[:], s_bc[:])
            nc.sync.dma_start(y.rearrange("(p j) -> p j", p=128), s_num[:])

    nc.compile()
    return nc


def _prep_inputs(inputs):
    f = lambda k: np.ascontiguousarray(np.asarray(inputs[k]), dtype=np.float32)
    x = f("x")
    centers, scales = f("centers"), f("scales")
    sub_W0, sub_b0 = f("sub_W0"), f("sub_b0")
    sub_Wmid, sub_bmid = f("sub_Wmid"), f("sub_bmid")
    sub_Wl, sub_bl = f("sub_Wl"), f("sub_bl")

    # fold per-subdomain normalization into layer-0 weights:
    # xn = (x - c)/s  =>  xn @ W0 + b0 = x @ (W0/s) + (b0 - (c/s) @ W0)
    w0e_full = sub_W0 / scales[:, :, None]                       # [C, D, SW]
    b0e_full = sub_b0 - np.einsum("cd,cdw->cw", centers / scales, sub_W0)

    w0e = np.ascontiguousarray(w0e_full.transpose(1, 0, 2).reshape(D, C * SW))
    b0e = np.ascontiguousarray(
        b0e_full.reshape(C, FC, 128).transpose(2, 0, 1).reshape(128, C * FC)
    )
    import ml_dtypes

    middt_np = ml_dtypes.bfloat16 if MID_BF16 else np.float32
    wm = np.ascontiguousarray(
        sub_Wmid.reshape(C, SNMID, KC, 128, FC, 128)
        .transpose(3, 0, 1, 4, 2, 5)
        .reshape(128, C * WBLK)
    ).astype(middt_np)
    bm = np.ascontiguousarray(
        sub_bmid.reshape(C, SNMID, FC, 128).transpose(3, 0, 1, 2).reshape(128, -1)
    )
    wl = np.ascontiguousarray(
        sub_Wl.reshape(C, KC, 128).transpose(2, 0, 1).reshape(128, -1)
    ).astype(middt_np)
    cwm = np.ascontiguousarray(
        np.stack([np.ones(C, np.float32), sub_bl[:, 0]], axis=1)
    )

    shared = dict(
        w0e=w0e,
        b0e=b0e,
        wm=wm,
        bm=bm,
        wl=wl,
        pw0=f("pou_W0"),
        pb0=np.ascontiguousarray(f("pou_b0")[:, None]),
        pwm=np.ascontiguousarray(f("pou_Wmid").transpose(1, 0, 2).reshape(PH, -1)),
        pbm=np.ascontiguousarray(f("pou_bmid").T),
        pwl=f("pou_Wl"),
        pbl=np.ascontiguousarray(f("pou_bl")[:, None]),
        cw=cwm,
    )

    in_maps = []
    for core in range(NCORES):
        xs = x[core * NP : (core + 1) * NP]
        m = dict(shared)
        m["xT"] = np.ascontiguousarray(xs.T)
        m["xP"] = np.ascontiguousarray(xs.reshape(128, 2 * PPB))
        in_maps.append(m)
    return in_maps


def kernel(**inputs):
    from concourse.bass_utils import run_bass_kernel_spmd

    if "nc" not in _CACHE:
        _CACHE["nc"] = _build()
    nc = _CACHE["nc"]

    in_maps = _prep_inputs(inputs)
    trace = os.environ.get("KERNEL_TRACE", "0") == "1"
    res = run_bass_kernel_spmd(
        nc, in_maps, core_ids=list(range(NCORES)), trace=trace
    )
    kernel.last_results = res
    y = np.concatenate([res.results[i]["y"] for i in range(NCORES)])
    return y.astype(np.float32)


# revision 7
# speedup vs baseline: 1.0471x; 1.0003x over previous
"""Trainium2 Bass kernel for FBPINN-with-window (dense MoE over 16 subnets).

Math (per point n):
    h   = relu(x @ pW0 + pb0); h += relu(h @ pWmid_l + pbmid_l) (x2)
    z   = h @ pWl + pbl;  ez = exp(z)            (softmax un-normalized)
    xn_c = (x - center_c)/scale_c  (folded on host into layer-0 weights)
    g_c = tanh(xn_c @ W0_c + b0_c); g_c = tanh(g_c @ Wmid_cl + bmid_cl) (x2)
    u_c = g_c @ Wl_c + bl_c
    acc = sum_c softmax(z)_c * u_c = (sum_c ez_c*(g_c@Wl_c) + sum_c ez_c*bl_c) / sum_c ez_c
    out = acc * x0(1-x0)*x1(1-x1)

Device layout: activations transposed (features on partitions, points on the
free dim).  Data-parallel over 8 cores (8192 points each), 4 point-tiles of
2048 per core.  Matmuls run as float32r (full fp32 storage, reduced-precision
multiply at full PE speed).  Mid-layer weights are streamed from HBM per
(tile, subnet) — they don't fit in SBUF alongside the activations.
"""

import os

import numpy as np

N = 65536
D = 2
C = 16
PH = 128
PNMID = 2
SW = 256
SNMID = 2

NCORES = 8
NP = N // NCORES          # 8192 points per core
PTILE = 2048              # points per tile
NT = NP // PTILE          # 4 tiles
CHK = 512                 # matmul moving free dim (one PSUM bank)
NCH = PTILE // CHK        # 4 chunks per tile
FC = SW // 128            # 2 feature chunks
KC = SW // 128            # 2 contraction chunks
PPB = NP // 128           # 64 points per partition (points-layout)
WBLK = SNMID * FC * KC * 128  # mid-weight cols per subnet (1024)

MID_BF16 = os.environ.get("KERNEL_BF16", "0") == "1"

_CACHE = {}


def _enable_ldw_opt():
    """walrus ships with --enable-ldw-opt=false; our matmul stream reuses each
    stationary operand across 4 consecutive matmuls, so the redundant-ldweights
    pass is a large win.  Patch the flag at the run_command boundary."""
    from concourse import bass_utils

    if getattr(bass_utils, "_ldw_opt_patched", False):
        return
    orig = bass_utils.run_command

    def patched(argv, **kw):
        argv = [
            "--enable-ldw-opt=true" if a == "--enable-ldw-opt=false" else a
            for a in argv
        ]
        return orig(argv, **kw)

    bass_utils.run_command = patched
    bass_utils._ldw_opt_patched = True


def _build():
    import concourse.mybir as mybir
    import concourse.tile as tile
    from concourse import bacc

    f32 = mybir.dt.float32
    f32r = mybir.dt.float32r
    bf16 = mybir.dt.bfloat16
    AF = mybir.ActivationFunctionType
    OP = mybir.AluOpType

    nc = bacc.Bacc("TRN2", debug=False)

    def din(name, shape, dt=f32):
        return nc.dram_tensor(name, shape, dt, kind="ExternalInput").ap()

    xT = din("xT", (D, NP), f32r)
    xP = din("xP", (128, 2 * PPB))
    w0e = din("w0e", (D, C * SW), f32r)
    b0e = din("b0e", (128, C * FC))
    middt = bf16 if MID_BF16 else f32r
    wm = din("wm", (128, C * WBLK), middt)
    bm = din("bm", (128, C * SNMID * FC))
    wl = din("wl", (128, C * KC), middt)
    pw0 = din("pw0", (D, PH), f32r)
    pb0 = din("pb0", (PH, 1))
    pwm = din("pwm", (PH, PNMID * PH), f32r)
    pbm = din("pbm", (PH, PNMID))
    pwl = din("pwl", (PH, C), f32r)
    pbl = din("pbl", (C, 1))
    cw = din("cw", (C, 2), f32r)
    y = nc.dram_tensor("y", (NP,), f32, kind="ExternalOutput").ap()

    with tile.TileContext(nc) as tc:
        with (
            tc.tile_pool(name="wp", bufs=1) as wp,
            tc.tile_pool(name="wmp", bufs=3) as wmp,
            tc.tile_pool(name="gp", bufs=3) as gp,
            tc.tile_pool(name="hp", bufs=2) as hp,
            tc.tile_pool(name="sp", bufs=2) as sp,
            tc.tile_pool(name="rp", bufs=3) as rp,
            tc.tile_pool(name="xp", bufs=2) as xpl,
            tc.tile_pool(name="fin", bufs=1) as fin,
            tc.tile_pool(name="pp", bufs=2, space="PSUM") as pp,
        ):
            # ---- small weights / constants into SBUF (resident) ----
            s_pw0 = wp.tile([D, PH], f32r)
            nc.sync.dma_start(s_pw0[:], pw0)
            s_pb0 = wp.tile([PH, 1], f32)
            nc.sync.dma_start(s_pb0[:], pb0)
            s_pwm = wp.tile([PH, PNMID * PH], f32r)
            nc.sync.dma_start(s_pwm[:], pwm)
            s_pbm = wp.tile([PH, PNMID], f32)
            nc.sync.dma_start(s_pbm[:], pbm)
            s_pwl = wp.tile([PH, C], f32r)
            nc.sync.dma_start(s_pwl[:], pwl)
            s_pbl = wp.tile([C, 1], f32)
            nc.sync.dma_start(s_pbl[:], pbl)
            s_cw = wp.tile([C, 2], f32r)
            nc.sync.dma_start(s_cw[:], cw)
            s_w0e = wp.tile([D, C * SW], f32r)
            nc.sync.dma_start(s_w0e[:], w0e)
            s_b0e = wp.tile([128, C * FC], f32)
            nc.sync.dma_start(s_b0e[:], b0e)
            s_wl = wp.tile([128, C * KC], middt)
            nc.sync.dma_start(s_wl[:], wl)
            s_bm = wp.tile([128, C * SNMID * FC], f32)
            nc.sync.dma_start(s_bm[:], bm)

            # ---- per-core x (points-layout) + boundary factor ----
            s_xP = fin.tile([128, 2 * PPB], f32)
            nc.sync.dma_start(s_xP[:], xP)
            s_xmx = fin.tile([128, 2 * PPB], f32)
            nc.vector.tensor_mul(s_xmx[:], s_xP[:], s_xP[:])
            nc.vector.tensor_sub(s_xmx[:], s_xP[:], s_xmx[:])
            v = s_xmx.rearrange("p (j two) -> p j two", two=2)
            s_bc = fin.tile([128, PPB], f32)
            nc.vector.tensor_mul(s_bc[:], v[:, :, 0], v[:, :, 1])

            # points-layout accumulators, filled per tile via reshape DMAs
            s_accP = fin.tile([128, PPB], f32)
            s_s1P = fin.tile([128, PPB], f32)
            s_s2P = fin.tile([128, PPB], f32)

            for t in range(NT):
                toff = t * PTILE
                s_xt = xpl.tile([D, PTILE], f32r, tag="xt")
                nc.sync.dma_start(s_xt[:], xT[:, toff : toff + PTILE])

                def xchunk(n):
                    return s_xt[:, n * CHK : (n + 1) * CHK]

                # ---------- PoU gating net ----------
                ps0 = pp.tile([PH, PTILE], f32, tag="mm")
                for n in range(NCH):
                    nc.tensor.matmul(
                        ps0[:, n * CHK : (n + 1) * CHK],
                        s_pw0[:],
                        xchunk(n),
                        start=True,
                        stop=True,
                    )
                h = hp.tile([PH, PTILE], f32r, tag="h")
                nc.vector.tensor_scalar(
                    h[:], ps0[:], s_pb0[:, 0:1], 0.0, op0=OP.add, op1=OP.max
                )
                for l in range(PNMID):
                    psl = pp.tile([PH, PTILE], f32, tag="mm")
                    for n in range(NCH):
                        nc.tensor.matmul(
                            psl[:, n * CHK : (n + 1) * CHK],
                            s_pwm[:, l * PH : (l + 1) * PH],
                            h[:, n * CHK : (n + 1) * CHK],
                            start=True,
                            stop=True,
                        )
                    hr = hp.tile([PH, PTILE], f32r, tag="h")
                    nc.vector.tensor_scalar(
                        hr[:], psl[:], s_pbm[:, l : l + 1], 0.0, op0=OP.add, op1=OP.max
                    )
                    nc.vector.tensor_add(hr[:], hr[:], h[:])
                    h = hr
                # logits -> ez
                psz = pp.tile([C, PTILE], f32, tag="mm")
                for n in range(NCH):
                    nc.tensor.matmul(
                        psz[:, n * CHK : (n + 1) * CHK],
                        s_pwl[:],
                        h[:, n * CHK : (n + 1) * CHK],
                        start=True,
                        stop=True,
                    )
                ez = sp.tile([C, PTILE], f32r, tag="ez")
                nc.scalar.activation(ez[:], psz[:], AF.Exp, bias=s_pbl[:, 0:1])
                # s1 = sum_c ez ; s2 = sum_c ez*bl_c
                pss = pp.tile([2, PTILE], f32, tag="mm")
                for n in range(NCH):
                    nc.tensor.matmul(
                        pss[:, n * CHK : (n + 1) * CHK],
                        s_cw[:],
                        ez[:, n * CHK : (n + 1) * CHK],
                        start=True,
                        stop=True,
                    )
                s12row = rp.tile([2, PTILE], f32, tag="row")
                nc.vector.tensor_copy(s12row[:], pss[:])
                nc.sync.dma_start(
                    s_s1P[t * 32 : (t + 1) * 32, :], s12row[0:1, :]
                )
                nc.sync.dma_start(
                    s_s2P[t * 32 : (t + 1) * 32, :], s12row[1:2, :]
                )

                # ---------- subnets (two chains interleaved) ----------
                u_asm = sp.tile([C, PTILE], f32r, tag="ua")

                def sub_dma(c, st):
                    s_wmc = wmp.tile([128, WBLK], middt, tag="wm")
                    nc.sync.dma_start(
                        s_wmc[:, : WBLK // 2],
                        wm[:, c * WBLK : c * WBLK + WBLK // 2],
                    )
                    nc.sync.dma_start(
                        s_wmc[:, WBLK // 2 :],
                        wm[:, c * WBLK + WBLK // 2 : (c + 1) * WBLK],
                    )
                    st["wm"] = s_wmc

                def sub_l0(c, st, fc):
                    if fc == 0:
                        st["g0"] = gp.tile([128, KC, PTILE], middt, tag="g", name="g0")
                    pt = pp.tile([128, PTILE], f32, tag="mm")
                    for n in range(NCH):
                        nc.tensor.matmul(
                            pt[:, n * CHK : (n + 1) * CHK],
                            s_w0e[:, c * SW + fc * 128 : c * SW + fc * 128 + 128],
                            xchunk(n),
                            start=True,
                            stop=True,
                        )
                    nc.scalar.activation(
                        st["g0"][:, fc, :],
                        pt[:],
                        AF.Tanh,
                        bias=s_b0e[:, c * FC + fc : c * FC + fc + 1],
                    )

                def sub_mid(c, st, l, fc):
                    gcur = st[f"g{l}"]
                    if fc == 0:
                        st[f"g{l + 1}"] = gp.tile([128, KC, PTILE], middt, tag="g", name=f"g{l + 1}")
                    pt = pp.tile([128, PTILE], f32, tag="mm")
                    for kc in range(KC):
                        col = ((l * FC + fc) * KC + kc) * 128
                        for n in range(NCH):
                            nc.tensor.matmul(
                                pt[:, n * CHK : (n + 1) * CHK],
                                st["wm"][:, col : col + 128],
                                gcur[:, kc, n * CHK : (n + 1) * CHK],
                                start=(kc == 0),
                                stop=(kc == KC - 1),
                            )
                    bcol = (c * SNMID + l) * FC + fc
                    nc.scalar.activation(
                        st[f"g{l + 1}"][:, fc, :],
                        pt[:],
                        AF.Tanh,
                        bias=s_bm[:, bcol : bcol + 1],
                    )

                def sub_last(c, st):
                    gcur = st[f"g{SNMID}"]
                    pu = pp.tile([1, PTILE], f32, tag="mm")
                    for kc in range(KC):
                        wcol = c * KC + kc
                        for n in range(NCH):
                            nc.tensor.matmul(
                                pu[:, n * CHK : (n + 1) * CHK],
                                s_wl[:, wcol : wcol + 1],
                                gcur[:, kc, n * CHK : (n + 1) * CHK],
                                start=(kc == 0),
                                stop=(kc == KC - 1),
                            )
                    urow = rp.tile([2, PTILE], f32r, tag="row")
                    nc.vector.tensor_copy(urow[0:1, :], pu[:])
                    nc.sync.dma_start(u_asm[c : c + 1, :], urow[0:1, :])

                def sub_step(c, st, k):
                    if k == 0:
                        sub_dma(c, st)
                    elif k <= 2:
                        sub_l0(c, st, k - 1)
                    elif k <= 6:
                        sub_mid(c, st, (k - 3) // FC, (k - 3) % FC)
                    else:
                        sub_last(c, st)

                for pair in range(C // 2):
                    ca, cb = 2 * pair, 2 * pair + 1
                    sa, sb = {}, {}
                    for k in range(8):
                        sub_step(ca, sa, k)
                        sub_step(cb, sb, k)

                # ---------- windowed combine ----------
                nc.vector.tensor_mul(u_asm[:], u_asm[:], ez[:])
                pacc = pp.tile([1, PTILE], f32, tag="mm")
                for n in range(NCH):
                    nc.tensor.matmul(
                        pacc[:, n * CHK : (n + 1) * CHK],
                        s_cw[:, 0:1],
                        u_asm[:, n * CHK : (n + 1) * CHK],
                        start=True,
                        stop=True,
                    )
                accrow = rp.tile([2, PTILE], f32, tag="row")
                nc.vector.tensor_copy(accrow[0:1, :], pacc[:])
                nc.sync.dma_start(
                    s_accP[t * 32 : (t + 1) * 32, :], accrow[0:1, :]
                )

            # ---------- final: combine in points-layout ----------
            s_r = fin.tile([128, PPB], f32)
            nc.vector.reciprocal(s_r[:], s_s1P[:])
            s_num = fin.tile([128, PPB], f32)
            nc.vector.tensor_add(s_num[:], s_accP[:], s_s2P[:])
            nc.vector.tensor_mul(s_num[:], s_num[:], s_r[:])
            nc.vector.tensor_mul(s_num[:], s_num[:], s_bc[:])
            nc.sync.dma_start(y.rearrange("(p j) -> p j", p=128), s_num[:])

    nc.compile()
    return nc


def _prep_inputs(inputs):
    f = lambda k: np.ascontiguousarray(np.asarray(inputs[k]), dtype=np.float32)
    x = f("x")
    centers, scales = f("centers"), f("scales")
    sub_W0, sub_b0 = f("sub_W0"), f("sub_b0")
    sub_Wmid, sub_bmid = f("sub_Wmid"), f("sub_bmid")
    sub_Wl, sub_bl = f("sub_Wl"), f("sub_bl")

    # fold per-subdomain normalization into layer-0 weights:
    # xn = (x - c)/s  =>  xn @ W0 + b0 = x @ (W0/s) + (b0 - (c/s) @ W0)
    w0e_full = sub_W0 / scales[:, :, None]                       # [C, D, SW]
    b0e_full = sub_b0 - np.einsum("cd,cdw->cw", centers / scales, sub_W0)

    w0e = np.ascontiguousarray(w0e_full.transpose(1, 0, 2).reshape(D, C * SW))
    b0e = np.ascontiguousarray(
        b0e_full.reshape(C, FC, 128).transpose(2, 0, 1).reshape(128, C * FC)
    )
    import ml_dtypes

    middt_np = ml_dtypes.bfloat16 if MID_BF16 else np.float32
    wm = np.ascontiguousarray(
        sub_Wmid.reshape(C, SNMID, KC, 128, FC, 128)
        .transpose(3, 0, 1, 4, 2, 5)
        .reshape(128, C * WBLK)
    ).astype(middt_np)
    bm = np.ascontiguousarray(
        sub_bmid.reshape(C, SNMID, FC, 128).transpose(3, 0, 1, 2).reshape(128, -1)
    )
    wl = np.ascontiguousarray(
        sub_Wl.reshape(C, KC, 128).transpose(2, 0, 1).reshape(128, -1)
    ).astype(middt_np)
    cwm = np.ascontiguousarray(
        np.stack([np.ones(C, np.float32), sub_bl[:, 0]], axis=1)
    )

    shared = dict(
        w0e=w0e,
        b0e=b0e,
        wm=wm,
        bm=bm,
        wl=wl,
        pw0=f("pou_W0"),
        pb0=np.ascontiguousarray(f("pou_b0")[:, None]),
        pwm=np.ascontiguousarray(f("pou_Wmid").transpose(1, 0, 2).reshape(PH, -1)),
        pbm=np.ascontiguousarray(f("pou_bmid").T),
        pwl=f("pou_Wl"),
        pbl=np.ascontiguousarray(f("pou_bl")[:, None]),
        cw=cwm,
    )

    in_maps = []
    for core in range(NCORES):
        xs = x[core * NP : (core + 1) * NP]
        m = dict(shared)
        m["xT"] = np.ascontiguousarray(xs.T)
        m["xP"] = np.ascontiguousarray(xs.reshape(128, 2 * PPB))
        in_maps.append(m)
    return in_maps


def kernel(**inputs):
    from concourse.bass_utils import run_bass_kernel_spmd

    if "nc" not in _CACHE:
        _enable_ldw_opt()
        _CACHE["nc"] = _build()
    nc = _CACHE["nc"]

    in_maps = _prep_inputs(inputs)
    trace = os.environ.get("KERNEL_TRACE", "0") == "1"
    res = run_bass_kernel_spmd(
        nc, in_maps, core_ids=list(range(NCORES)), trace=trace
    )
    kernel.last_results = res
    y = np.concatenate([res.results[i]["y"] for i in range(NCORES)])
    return y.astype(np.float32)


# revision 11
# speedup vs baseline: 1.1109x; 1.0609x over previous
"""Trainium2 Bass kernel for FBPINN-with-window (dense MoE over 16 subnets).

Math (per point n):
    h   = relu(x @ pW0 + pb0); h += relu(h @ pWmid_l + pbmid_l) (x2)
    z   = h @ pWl + pbl;  ez = exp(z)            (softmax un-normalized)
    xn_c = (x - center_c)/scale_c  (folded on host into layer-0 weights)
    g_c = tanh(xn_c @ W0_c + b0_c); g_c = tanh(g_c @ Wmid_cl + bmid_cl) (x2)
    u_c = g_c @ Wl_c + bl_c
    acc = sum_c softmax(z)_c * u_c = (sum_c ez_c*(g_c@Wl_c) + sum_c ez_c*bl_c) / sum_c ez_c
    out = acc * x0(1-x0)*x1(1-x1)

Device layout: activations transposed (features on partitions, points on the
free dim).  Data-parallel over 8 cores (8192 points each), 4 point-tiles of
2048 per core.  Matmuls run as float32r (full fp32 storage, reduced-precision
multiply at full PE speed).  Mid-layer weights are streamed from HBM per
(tile, subnet) — they don't fit in SBUF alongside the activations.
"""

import os

import numpy as np

N = 65536
D = 2
C = 16
PH = 128
PNMID = 2
SW = 256
SNMID = 2

NCORES = 8
NP = N // NCORES          # 8192 points per core
PTILE = 2048              # points per tile
NT = NP // PTILE          # 4 tiles
CHK = 512                 # matmul moving free dim (one PSUM bank)
NCH = PTILE // CHK        # 4 chunks per tile
FC = SW // 128            # 2 feature chunks
KC = SW // 128            # 2 contraction chunks
PPB = NP // 128           # 64 points per partition (points-layout)
WBLK = SNMID * FC * KC * 128  # mid-weight cols per subnet (1024)

MID_BF16 = os.environ.get("KERNEL_BF16", "0") == "1"

_CACHE = {}


def _enable_ldw_opt():
    """walrus ships with --enable-ldw-opt=false; our matmul stream reuses each
    stationary operand across 4 consecutive matmuls, so the redundant-ldweights
    pass is a large win.  Patch the flag at the run_command boundary."""
    from concourse import bass_utils

    if getattr(bass_utils, "_ldw_opt_patched", False):
        return
    orig = bass_utils.run_command

    def patched(argv, **kw):
        argv = [
            "--enable-ldw-opt=true" if a == "--enable-ldw-opt=false" else a
            for a in argv
        ]
        return orig(argv, **kw)

    bass_utils.run_command = patched
    bass_utils._ldw_opt_patched = True


def _build():
    import concourse.mybir as mybir
    import concourse.tile as tile
    from concourse import bacc

    f32 = mybir.dt.float32
    f32r = mybir.dt.float32r
    bf16 = mybir.dt.bfloat16
    AF = mybir.ActivationFunctionType
    OP = mybir.AluOpType

    nc = bacc.Bacc("TRN2", debug=False)

    def din(name, shape, dt=f32):
        return nc.dram_tensor(name, shape, dt, kind="ExternalInput").ap()

    xT = din("xT", (3, NP), f32r)
    xP = din("xP", (128, 2 * PPB))
    w0q = din("w0q", (128, C * FC * 128), f32r)
    middt = bf16 if MID_BF16 else f32r
    wm = din("wm", (128, C * WBLK), middt)
    bm = din("bm", (128, C * SNMID * FC))
    wl = din("wl", (128, C * KC), middt)
    pw0q = din("pw0q", (128, PH), f32r)
    pwm = din("pwm", (PH, PNMID * PH), f32r)
    pbm = din("pbm", (PH, PNMID))
    pwl = din("pwl", (PH, C), f32r)
    pbl = din("pbl", (C, 1))
    cw = din("cw", (C, 2), f32r)
    y = nc.dram_tensor("y", (NP,), f32, kind="ExternalOutput").ap()

    with tile.TileContext(nc) as tc:
        with (
            tc.tile_pool(name="wp", bufs=1) as wp,
            tc.tile_pool(name="wmp", bufs=3) as wmp,
            tc.tile_pool(name="gp", bufs=3) as gp,
            tc.tile_pool(name="hp", bufs=2) as hp,
            tc.tile_pool(name="sp", bufs=2) as sp,
            tc.tile_pool(name="rp", bufs=3) as rp,
            tc.tile_pool(name="xp", bufs=2) as xpl,
            tc.tile_pool(name="fin", bufs=1) as fin,
            tc.tile_pool(name="pp", bufs=2, space="PSUM") as pp,
        ):
            # ---- small weights / constants into SBUF (resident) ----
            s_pw0q = wp.tile([128, PH], f32r)
            nc.sync.dma_start(s_pw0q[:], pw0q)
            s_pwm = wp.tile([PH, PNMID * PH], f32r)
            nc.sync.dma_start(s_pwm[:], pwm)
            s_pbm = wp.tile([PH, PNMID], f32)
            nc.sync.dma_start(s_pbm[:], pbm)
            s_pwl = wp.tile([PH, C], f32r)
            nc.sync.dma_start(s_pwl[:], pwl)
            s_pbl = wp.tile([C, 1], f32)
            nc.sync.dma_start(s_pbl[:], pbl)
            s_cw = wp.tile([C, 2], f32r)
            nc.sync.dma_start(s_cw[:], cw)
            s_w0q = wp.tile([128, C * FC * 128], f32r)
            for cq in range(4):
                qb = C * FC * 32
                nc.sync.dma_start(
                    s_w0q[:, cq * qb : (cq + 1) * qb], w0q[:, cq * qb : (cq + 1) * qb]
                )
            s_wl = wp.tile([128, C * KC], middt)
            nc.sync.dma_start(s_wl[:], wl)
            s_bm = wp.tile([128, C * SNMID * FC], f32)
            nc.sync.dma_start(s_bm[:], bm)

            # ---- per-core x (points-layout) + boundary factor ----
            s_xP = fin.tile([128, 2 * PPB], f32)
            nc.sync.dma_start(s_xP[:], xP)
            s_xmx = fin.tile([128, 2 * PPB], f32)
            nc.vector.tensor_mul(s_xmx[:], s_xP[:], s_xP[:])
            nc.vector.tensor_sub(s_xmx[:], s_xP[:], s_xmx[:])
            v = s_xmx.rearrange("p (j two) -> p j two", two=2)
            s_bc = fin.tile([128, PPB], f32)
            nc.vector.tensor_mul(s_bc[:], v[:, :, 0], v[:, :, 1])

            # points-layout accumulators, filled per tile via reshape DMAs
            s_accP = fin.tile([128, PPB], f32)
            s_s1P = fin.tile([128, PPB], f32)
            s_s2P = fin.tile([128, PPB], f32)

            for t in range(NT):
                toff = t * PTILE
                # x chunks replicated into 4 row-groups (rows 32r..32r+2 hold
                # [x0; x1; 1] for point-chunk r) so K=3 matmuls pack 4-wide.
                xt4 = xpl.tile([128, PTILE], f32r, tag="xt")
                for rr in range(NCH):
                    nc.sync.dma_start(
                        xt4[32 * rr : 32 * rr + 3, rr * CHK : (rr + 1) * CHK],
                        xT[:, toff + rr * CHK : toff + (rr + 1) * CHK],
                    )

                pst = {}

                def pou_l0():
                    ps0 = pp.tile([PH, PTILE], f32, tag="mm")
                    for rr in range(NCH):
                        nc.tensor.matmul(
                            ps0[:, rr * CHK : (rr + 1) * CHK],
                            s_pw0q[32 * rr : 32 * rr + 3, :],
                            xt4[32 * rr : 32 * rr + 3, rr * CHK : (rr + 1) * CHK],
                            start=True,
                            stop=True,
                            tile_position=(32 * rr, 0),
                        )
                    h = hp.tile([PH, PTILE], f32r, tag="h")
                    nc.vector.tensor_scalar_max(h[:], ps0[:], 0.0)
                    pst["h"] = h

                def pou_mid(l):
                    h = pst["h"]
                    psl = pp.tile([PH, PTILE], f32, tag="mm")
                    for n in range(NCH):
                        nc.tensor.matmul(
                            psl[:, n * CHK : (n + 1) * CHK],
                            s_pwm[:, l * PH : (l + 1) * PH],
                            h[:, n * CHK : (n + 1) * CHK],
                            start=True,
                            stop=True,
                        )
                    hr = hp.tile([PH, PTILE], f32r, tag="h")
                    nc.vector.tensor_scalar(
                        hr[:], psl[:], s_pbm[:, l : l + 1], 0.0, op0=OP.add, op1=OP.max
                    )
                    nc.vector.tensor_add(hr[:], hr[:], h[:])
                    pst["h"] = hr

                def pou_logits():
                    h = pst["h"]
                    psz = pp.tile([C, PTILE], f32, tag="mm")
                    for n in range(NCH):
                        nc.tensor.matmul(
                            psz[:, n * CHK : (n + 1) * CHK],
                            s_pwl[:],
                            h[:, n * CHK : (n + 1) * CHK],
                            start=True,
                            stop=True,
                        )
                    ez = sp.tile([C, PTILE], f32r, tag="ez")
                    nc.scalar.activation(ez[:], psz[:], AF.Exp, bias=s_pbl[:, 0:1])
                    pst["ez"] = ez

                def pou_s12():
                    ez = pst["ez"]
                    pss = pp.tile([2, PTILE], f32, tag="mm")
                    for n in range(NCH):
                        nc.tensor.matmul(
                            pss[:, n * CHK : (n + 1) * CHK],
                            s_cw[:],
                            ez[:, n * CHK : (n + 1) * CHK],
                            start=True,
                            stop=True,
                        )
                    s12row = rp.tile([2, PTILE], f32, tag="row")
                    nc.vector.tensor_copy(s12row[:], pss[:])
                    nc.sync.dma_start(
                        s_s1P[t * 32 : (t + 1) * 32, :], s12row[0:1, :]
                    )
                    nc.sync.dma_start(
                        s_s2P[t * 32 : (t + 1) * 32, :], s12row[1:2, :]
                    )

                pou_steps = [
                    pou_l0,
                    lambda: pou_mid(0),
                    lambda: pou_mid(1),
                    pou_logits,
                    pou_s12,
                ]

                # ---------- subnets (two chains interleaved) ----------
                u_asm = sp.tile([C, PTILE], f32r, tag="ua")

                def sub_dma(c, st):
                    s_wmc = wmp.tile([128, WBLK], middt, tag="wm")
                    nc.sync.dma_start(
                        s_wmc[:, : WBLK // 2],
                        wm[:, c * WBLK : c * WBLK + WBLK // 2],
                    )
                    nc.sync.dma_start(
                        s_wmc[:, WBLK // 2 :],
                        wm[:, c * WBLK + WBLK // 2 : (c + 1) * WBLK],
                    )
                    st["wm"] = s_wmc

                def sub_l0(c, st, fc):
                    if fc == 0:
                        st["g0"] = gp.tile([128, KC, PTILE], middt, tag="g", name="g0")
                    col = (c * FC + fc) * 128
                    pt = pp.tile([128, PTILE], f32, tag="mm")
                    for rr in range(NCH):
                        nc.tensor.matmul(
                            pt[:, rr * CHK : (rr + 1) * CHK],
                            s_w0q[32 * rr : 32 * rr + 3, col : col + 128],
                            xt4[32 * rr : 32 * rr + 3, rr * CHK : (rr + 1) * CHK],
                            start=True,
                            stop=True,
                            tile_position=(32 * rr, 0),
                        )
                    nc.scalar.activation(st["g0"][:, fc, :], pt[:], AF.Tanh)

                def sub_mid(c, st, l, fc):
                    gcur = st[f"g{l}"]
                    if fc == 0:
                        st[f"g{l + 1}"] = gp.tile(
                            [128, KC, PTILE], middt, tag="g", name=f"g{l + 1}"
                        )
                    pt = pp.tile([128, PTILE], f32, tag="mm")
                    for kc in range(KC):
                        col = ((l * FC + fc) * KC + kc) * 128
                        for n in range(NCH):
                            nc.tensor.matmul(
                                pt[:, n * CHK : (n + 1) * CHK],
                                st["wm"][:, col : col + 128],
                                gcur[:, kc, n * CHK : (n + 1) * CHK],
                                start=(kc == 0),
                                stop=(kc == KC - 1),
                            )
                    bcol = (c * SNMID + l) * FC + fc
                    nc.scalar.activation(
                        st[f"g{l + 1}"][:, fc, :],
                        pt[:],
                        AF.Tanh,
                        bias=s_bm[:, bcol : bcol + 1],
                    )

                def sub_last(c, st):
                    gcur = st[f"g{SNMID}"]
                    pu = pp.tile([1, PTILE], f32, tag="mm")
                    for kc in range(KC):
                        wcol = c * KC + kc
                        for n in range(NCH):
                            nc.tensor.matmul(
                                pu[:, n * CHK : (n + 1) * CHK],
                                s_wl[:, wcol : wcol + 1],
                                gcur[:, kc, n * CHK : (n + 1) * CHK],
                                start=(kc == 0),
                                stop=(kc == KC - 1),
                            )
                    urow = rp.tile([2, PTILE], f32r, tag="row")
                    nc.vector.tensor_copy(urow[0:1, :], pu[:])
                    nc.sync.dma_start(u_asm[c : c + 1, :], urow[0:1, :])

                def sub_step(c, st, k):
                    if k == 0:
                        sub_dma(c, st)
                    elif k <= 2:
                        sub_l0(c, st, k - 1)
                    elif k <= 6:
                        sub_mid(c, st, (k - 3) // FC, (k - 3) % FC)
                    else:
                        sub_last(c, st)

                for pair in range(C // 2):
                    ca, cb = 2 * pair, 2 * pair + 1
                    sa, sb = {}, {}
                    for k in range(8):
                        if pair == 0 and k < len(pou_steps):
                            pou_steps[k]()
                        sub_step(ca, sa, k)
                        sub_step(cb, sb, k)

                # ---------- windowed combine ----------
                ez = pst["ez"]
                nc.vector.tensor_mul(u_asm[:], u_asm[:], ez[:])
                pacc = pp.tile([1, PTILE], f32, tag="mm")
                for n in range(NCH):
                    nc.tensor.matmul(
                        pacc[:, n * CHK : (n + 1) * CHK],
                        s_cw[:, 0:1],
                        u_asm[:, n * CHK : (n + 1) * CHK],
                        start=True,
                        stop=True,
                    )
                accrow = rp.tile([2, PTILE], f32, tag="row")
                nc.vector.tensor_copy(accrow[0:1, :], pacc[:])
                nc.sync.dma_start(
                    s_accP[t * 32 : (t + 1) * 32, :], accrow[0:1, :]
                )

            # ---------- final: combine in points-layout ----------
            s_r = fin.tile([128, PPB], f32)
            nc.vector.reciprocal(s_r[:], s_s1P[:])
            s_num = fin.tile([128, PPB], f32)
            nc.vector.tensor_add(s_num[:], s_accP[:], s_s2P[:])
            nc.vector.tensor_mul(s_num[:], s_num[:], s_r[:])
            nc.vector.tensor_mul(s_num[:], s_num[:], s_bc[:])
            nc.sync.dma_start(y.rearrange("(p j) -> p j", p=128), s_num[:])

    nc.compile()
    return nc


def _prep_inputs(inputs):
    f = lambda k: np.ascontiguousarray(np.asarray(inputs[k]), dtype=np.float32)
    x = f("x")
    centers, scales = f("centers"), f("scales")
    sub_W0, sub_b0 = f("sub_W0"), f("sub_b0")
    sub_Wmid, sub_bmid = f("sub_Wmid"), f("sub_bmid")
    sub_Wl, sub_bl = f("sub_Wl"), f("sub_bl")

    # fold per-subdomain normalization into layer-0 weights:
    # xn = (x - c)/s  =>  xn @ W0 + b0 = x @ (W0/s) + (b0 - (c/s) @ W0)
    w0e_full = sub_W0 / scales[:, :, None]                       # [C, D, SW]
    b0e_full = sub_b0 - np.einsum("cd,cdw->cw", centers / scales, sub_W0)

    # row-group packed layer-0 weights: rows {32r,32r+1,32r+2} = [W0; W1; b]
    w0q = np.zeros((128, C * FC * 128), np.float32)
    for c in range(C):
        for fc in range(FC):
            col = (c * FC + fc) * 128
            blk = np.vstack(
                [
                    w0e_full[c][:, fc * 128 : (fc + 1) * 128],
                    b0e_full[c][None, fc * 128 : (fc + 1) * 128],
                ]
            )
            for rr in range(4):
                w0q[32 * rr : 32 * rr + 3, col : col + 128] = blk

    import ml_dtypes

    middt_np = ml_dtypes.bfloat16 if MID_BF16 else np.float32
    wm = np.ascontiguousarray(
        sub_Wmid.reshape(C, SNMID, KC, 128, FC, 128)
        .transpose(3, 0, 1, 4, 2, 5)
        .reshape(128, C * WBLK)
    ).astype(middt_np)
    bm = np.ascontiguousarray(
        sub_bmid.reshape(C, SNMID, FC, 128).transpose(3, 0, 1, 2).reshape(128, -1)
    )
    wl = np.ascontiguousarray(
        sub_Wl.reshape(C, KC, 128).transpose(2, 0, 1).reshape(128, -1)
    ).astype(middt_np)
    cwm = np.ascontiguousarray(
        np.stack([np.ones(C, np.float32), sub_bl[:, 0]], axis=1)
    )

    pw0q = np.zeros((128, PH), np.float32)
    pblk = np.vstack([f("pou_W0"), f("pou_b0")[None, :]])
    for rr in range(4):
        pw0q[32 * rr : 32 * rr + 3, :] = pblk

    shared = dict(
        w0q=w0q,
        wm=wm,
        bm=bm,
        wl=wl,
        pw0q=pw0q,
        pwm=np.ascontiguousarray(f("pou_Wmid").transpose(1, 0, 2).reshape(PH, -1)),
        pbm=np.ascontiguousarray(f("pou_bmid").T),
        pwl=f("pou_Wl"),
        pbl=np.ascontiguousarray(f("pou_bl")[:, None]),
        cw=cwm,
    )

    in_maps = []
    for core in range(NCORES):
        xs = x[core * NP : (core + 1) * NP]
        m = dict(shared)
        m["xT"] = np.ascontiguousarray(
            np.vstack([xs.T, np.ones((1, NP), np.float32)])
        )
        m["xP"] = np.ascontiguousarray(xs.reshape(128, 2 * PPB))
        in_maps.append(m)
    return in_maps


def kernel(**inputs):
    from concourse.bass_utils import run_bass_kernel_spmd

    if "nc" not in _CACHE:
        _enable_ldw_opt()
        _CACHE["nc"] = _build()
    nc = _CACHE["nc"]

    in_maps = _prep_inputs(inputs)
    trace = os.environ.get("KERNEL_TRACE", "0") == "1"
    res = run_bass_kernel_spmd(
        nc, in_maps, core_ids=list(range(NCORES)), trace=trace
    )
    kernel.last_results = res
    y = np.concatenate([res.results[i]["y"] for i in range(NCORES)])
    return y.astype(np.float32)


# revision 13
# speedup vs baseline: 1.2778x; 1.1503x over previous
"""Trainium2 Bass kernel for FBPINN-with-window (dense MoE over 16 subnets).

Math (per point n):
    h   = relu(x @ pW0 + pb0); h += relu(h @ pWmid_l + pbmid_l) (x2)
    z   = h @ pWl + pbl;  ez = exp(z)            (softmax un-normalized)
    xn_c = (x - center_c)/scale_c  (folded on host into layer-0 weights)
    g_c = tanh(xn_c @ W0_c + b0_c); g_c = tanh(g_c @ Wmid_cl + bmid_cl) (x2)
    u_c = g_c @ Wl_c + bl_c
    acc = sum_c softmax(z)_c * u_c = (sum_c ez_c*(g_c@Wl_c) + sum_c ez_c*bl_c) / sum_c ez_c
    out = acc * x0(1-x0)*x1(1-x1)

Device layout: activations transposed (features on partitions, points on the
free dim).  Data-parallel over 8 cores (8192 points each), 4 point-tiles of
2048 per core.  Matmuls run as float32r (full fp32 storage, reduced-precision
multiply at full PE speed).  Mid-layer weights are streamed from HBM per
(tile, subnet) — they don't fit in SBUF alongside the activations.
"""

import os

import numpy as np

N = 65536
D = 2
C = 16
PH = 128
PNMID = 2
SW = 256
SNMID = 2

NCORES = 8
NP = N // NCORES          # 8192 points per core
PTILE = 2048              # points per tile
NT = NP // PTILE          # 4 tiles
CHK = 512                 # matmul moving free dim (one PSUM bank)
NCH = PTILE // CHK        # 4 chunks per tile
FC = SW // 128            # 2 feature chunks
KC = SW // 128            # 2 contraction chunks
PPB = NP // 128           # 64 points per partition (points-layout)
WBLK = SNMID * FC * KC * 128  # mid-weight cols per subnet (1024)

MID_BF16 = os.environ.get("KERNEL_BF16", "0") == "1"

_CACHE = {}


def _enable_ldw_opt():
    """walrus ships with --enable-ldw-opt=false; our matmul stream reuses each
    stationary operand across 4 consecutive matmuls, so the redundant-ldweights
    pass is a large win.  Patch the flag at the run_command boundary."""
    from concourse import bass_utils

    if getattr(bass_utils, "_ldw_opt_patched", False):
        return
    orig = bass_utils.run_command

    def patched(argv, **kw):
        argv = [
            "--enable-ldw-opt=true" if a == "--enable-ldw-opt=false" else a
            for a in argv
        ]
        return orig(argv, **kw)

    bass_utils.run_command = patched
    bass_utils._ldw_opt_patched = True


def _build():
    import concourse.mybir as mybir
    import concourse.tile as tile
    from concourse import bacc

    f32 = mybir.dt.float32
    f32r = mybir.dt.float32r
    bf16 = mybir.dt.bfloat16
    AF = mybir.ActivationFunctionType
    OP = mybir.AluOpType

    nc = bacc.Bacc("TRN2", debug=False)

    def din(name, shape, dt=f32):
        return nc.dram_tensor(name, shape, dt, kind="ExternalInput").ap()

    xT = din("xT", (3, NP), f32r)
    xP = din("xP", (128, 2 * PPB))
    w0q = din("w0q", (128, C * FC * 128), f32r)
    middt = bf16 if MID_BF16 else f32r
    wm = din("wm", (128, C * WBLK), middt)
    bm = din("bm", (128, C * SNMID * FC))
    wl = din("wl", (128, C * KC), middt)
    pw0q = din("pw0q", (128, PH), f32r)
    pwm = din("pwm", (PH, PNMID * PH), f32r)
    pbm = din("pbm", (PH, PNMID))
    pwl = din("pwl", (PH, C), f32r)
    pbl = din("pbl", (C, 1))
    cw = din("cw", (C, 2), f32r)
    y = nc.dram_tensor("y", (NP,), f32, kind="ExternalOutput").ap()

    with tile.TileContext(nc) as tc:
        with (
            tc.tile_pool(name="wp", bufs=1) as wp,
            tc.tile_pool(name="wmp", bufs=4) as wmp,
            tc.tile_pool(name="gp", bufs=4) as gp,
            tc.tile_pool(name="hp", bufs=2) as hp,
            tc.tile_pool(name="sp", bufs=2) as sp,
            tc.tile_pool(name="rp", bufs=2) as rp,
            tc.tile_pool(name="xp", bufs=2) as xpl,
            tc.tile_pool(name="fin", bufs=1) as fin,
            tc.tile_pool(name="pp", bufs=2, space="PSUM") as pp,
        ):
            # ---- small weights / constants into SBUF (resident) ----
            s_pw0q = wp.tile([128, PH], f32r)
            nc.sync.dma_start(s_pw0q[:], pw0q)
            s_pwm = wp.tile([PH, PNMID * PH], f32r)
            nc.sync.dma_start(s_pwm[:], pwm)
            s_pbm = wp.tile([PH, PNMID], f32)
            nc.sync.dma_start(s_pbm[:], pbm)
            s_pwl = wp.tile([PH, C], f32r)
            nc.sync.dma_start(s_pwl[:], pwl)
            s_pbl = wp.tile([C, 1], f32)
            nc.sync.dma_start(s_pbl[:], pbl)
            s_cw = wp.tile([C, 2], f32r)
            nc.sync.dma_start(s_cw[:], cw)
            s_w0q = wp.tile([128, C * FC * 128], f32r)
            for cq in range(4):
                qb = C * FC * 32
                nc.sync.dma_start(
                    s_w0q[:, cq * qb : (cq + 1) * qb], w0q[:, cq * qb : (cq + 1) * qb]
                )
            s_wl = wp.tile([128, C * KC], middt)
            nc.sync.dma_start(s_wl[:], wl)
            s_bm = wp.tile([128, C * SNMID * FC], f32)
            nc.sync.dma_start(s_bm[:], bm)

            # ---- per-core x (points-layout) + boundary factor ----
            s_xP = fin.tile([128, 2 * PPB], f32)
            nc.sync.dma_start(s_xP[:], xP)
            s_xmx = fin.tile([128, 2 * PPB], f32)
            nc.vector.tensor_mul(s_xmx[:], s_xP[:], s_xP[:])
            nc.vector.tensor_sub(s_xmx[:], s_xP[:], s_xmx[:])
            v = s_xmx.rearrange("p (j two) -> p j two", two=2)
            s_bc = fin.tile([128, PPB], f32)
            nc.vector.tensor_mul(s_bc[:], v[:, :, 0], v[:, :, 1])

            # points-layout accumulators, filled per tile via reshape DMAs
            s_accP = fin.tile([128, PPB], f32)
            s_s1P = fin.tile([128, PPB], f32)
            s_s2P = fin.tile([128, PPB], f32)

            def wm_fetch(c):
                s_wmc = wmp.tile([128, WBLK], middt, tag="wm", name="wmc")
                q = WBLK // 4
                for piece in range(4):
                    nc.sync.dma_start(
                        s_wmc[:, piece * q : (piece + 1) * q],
                        wm[:, c * WBLK + piece * q : c * WBLK + (piece + 1) * q],
                    )
                return s_wmc

            wm_tiles = {}
            xt4_next = {}
            for t in range(NT):
                toff = t * PTILE
                if t == 0:
                    wm_tiles[0] = wm_fetch(0)
                    wm_tiles[1] = wm_fetch(1)
                    # x chunks replicated into 4 row-groups (rows 32r..32r+2
                    # hold [x0; x1; 1] for point-chunk r): K=3 packs 4-wide.
                    xt4 = xpl.tile([128, PTILE], f32r, tag="xt")
                    for rr in range(NCH):
                        nc.sync.dma_start(
                            xt4[32 * rr : 32 * rr + 3, rr * CHK : (rr + 1) * CHK],
                            xT[:, toff + rr * CHK : toff + (rr + 1) * CHK],
                        )
                else:
                    xt4 = xt4_next.pop(0)

                pst = {}

                def pou_l0():
                    ps0 = pp.tile([PH, PTILE], f32, tag="mm")
                    for rr in range(NCH):
                        nc.tensor.matmul(
                            ps0[:, rr * CHK : (rr + 1) * CHK],
                            s_pw0q[32 * rr : 32 * rr + 3, :],
                            xt4[32 * rr : 32 * rr + 3, rr * CHK : (rr + 1) * CHK],
                            start=True,
                            stop=True,
                            tile_position=(32 * rr, 0),
                        )
                    h = hp.tile([PH, PTILE], f32r, tag="h")
                    nc.vector.tensor_scalar_max(h[:], ps0[:], 0.0)
                    pst["h"] = h

                def pou_mid(l):
                    h = pst["h"]
                    psl = pp.tile([PH, PTILE], f32, tag="mm")
                    for n in range(NCH):
                        nc.tensor.matmul(
                            psl[:, n * CHK : (n + 1) * CHK],
                            s_pwm[:, l * PH : (l + 1) * PH],
                            h[:, n * CHK : (n + 1) * CHK],
                            start=True,
                            stop=True,
                        )
                    hr = hp.tile([PH, PTILE], f32r, tag="h")
                    nc.vector.tensor_scalar(
                        hr[:], psl[:], s_pbm[:, l : l + 1], 0.0, op0=OP.add, op1=OP.max
                    )
                    nc.vector.tensor_add(hr[:], hr[:], h[:])
                    pst["h"] = hr

                def pou_logits():
                    h = pst["h"]
                    psz = pp.tile([C, PTILE], f32, tag="mm")
                    for n in range(NCH):
                        nc.tensor.matmul(
                            psz[:, n * CHK : (n + 1) * CHK],
                            s_pwl[:],
                            h[:, n * CHK : (n + 1) * CHK],
                            start=True,
                            stop=True,
                        )
                    ez = sp.tile([C, PTILE], f32r, tag="ez")
                    nc.scalar.activation(ez[:], psz[:], AF.Exp, bias=s_pbl[:, 0:1])
                    pst["ez"] = ez

                def pou_s12():
                    ez = pst["ez"]
                    pss = pp.tile([2, PTILE], f32, tag="mm")
                    for n in range(NCH):
                        nc.tensor.matmul(
                            pss[:, n * CHK : (n + 1) * CHK],
                            s_cw[:],
                            ez[:, n * CHK : (n + 1) * CHK],
                            start=True,
                            stop=True,
                        )
                    s12row = rp.tile([2, PTILE], f32, tag="row")
                    nc.vector.tensor_copy(s12row[:], pss[:])
                    nc.sync.dma_start(
                        s_s1P[t * 32 : (t + 1) * 32, :], s12row[0:1, :]
                    )
                    nc.sync.dma_start(
                        s_s2P[t * 32 : (t + 1) * 32, :], s12row[1:2, :]
                    )

                pou_steps = [
                    pou_l0,
                    lambda: pou_mid(0),
                    lambda: pou_mid(1),
                    pou_logits,
                    pou_s12,
                ]

                # ---------- subnets (two chains interleaved) ----------
                u_asm = sp.tile([C, PTILE], f32r, tag="ua")

                def sub_l0(c, st, fc):
                    if fc == 0:
                        st["g0"] = gp.tile([128, KC, PTILE], middt, tag="g", name="g0")
                    col = (c * FC + fc) * 128
                    pt = pp.tile([128, PTILE], f32, tag="mm")
                    for rr in range(NCH):
                        nc.tensor.matmul(
                            pt[:, rr * CHK : (rr + 1) * CHK],
                            s_w0q[32 * rr : 32 * rr + 3, col : col + 128],
                            xt4[32 * rr : 32 * rr + 3, rr * CHK : (rr + 1) * CHK],
                            start=True,
                            stop=True,
                            tile_position=(32 * rr, 0),
                        )
                    nc.scalar.activation(st["g0"][:, fc, :], pt[:], AF.Tanh)

                def sub_mid(c, st, l, fc):
                    gcur = st[f"g{l}"]
                    if fc == 0:
                        st[f"g{l + 1}"] = gp.tile(
                            [128, KC, PTILE], middt, tag="g", name=f"g{l + 1}"
                        )
                    pt = pp.tile([128, PTILE], f32, tag="mm")
                    for kc in range(KC):
                        col = ((l * FC + fc) * KC + kc) * 128
                        for n in range(NCH):
                            nc.tensor.matmul(
                                pt[:, n * CHK : (n + 1) * CHK],
                                st["wm"][:, col : col + 128],
                                gcur[:, kc, n * CHK : (n + 1) * CHK],
                                start=(kc == 0),
                                stop=(kc == KC - 1),
                            )
                    bcol = (c * SNMID + l) * FC + fc
                    nc.scalar.activation(
                        st[f"g{l + 1}"][:, fc, :],
                        pt[:],
                        AF.Tanh,
                        bias=s_bm[:, bcol : bcol + 1],
                    )

                def sub_last(c, st):
                    gcur = st[f"g{SNMID}"]
                    pu = pp.tile([1, PTILE], f32, tag="mm")
                    for kc in range(KC):
                        wcol = c * KC + kc
                        for n in range(NCH):
                            nc.tensor.matmul(
                                pu[:, n * CHK : (n + 1) * CHK],
                                s_wl[:, wcol : wcol + 1],
                                gcur[:, kc, n * CHK : (n + 1) * CHK],
                                start=(kc == 0),
                                stop=(kc == KC - 1),
                            )
                    urow = rp.tile([2, PTILE], f32r, tag="row")
                    nc.vector.tensor_copy(urow[0:1, :], pu[:])
                    nc.sync.dma_start(u_asm[c : c + 1, :], urow[0:1, :])

                def sub_step(c, st, k):
                    if k <= 1:
                        sub_l0(c, st, k)
                    elif k <= 5:
                        sub_mid(c, st, (k - 2) // FC, (k - 2) % FC)
                    else:
                        sub_last(c, st)

                for pair in range(C // 2):
                    ca, cb = 2 * pair, 2 * pair + 1
                    # weights for this pair were prefetched one pair earlier
                    sa = {"wm": wm_tiles.pop(ca)}
                    sb = {"wm": wm_tiles.pop(cb)}
                    # prefetch next pair's weights (wraps into next tile's c0/c1)
                    if pair + 1 < C // 2:
                        wm_tiles[ca + 2] = wm_fetch(ca + 2)
                        wm_tiles[cb + 2] = wm_fetch(cb + 2)
                    else:
                        wm_tiles[0] = wm_fetch(0)
                        wm_tiles[1] = wm_fetch(1)
                    if pair == C // 2 - 2 and t + 1 < NT:
                        # prefetch next tile's x layout
                        nxt = xpl.tile([128, PTILE], f32r, tag="xt", name="xt4")
                        for rr in range(NCH):
                            nc.sync.dma_start(
                                nxt[32 * rr : 32 * rr + 3, rr * CHK : (rr + 1) * CHK],
                                xT[
                                    :,
                                    (t + 1) * PTILE
                                    + rr * CHK : (t + 1) * PTILE
                                    + (rr + 1) * CHK,
                                ],
                            )
                        xt4_next[0] = nxt
                    for k in range(7):
                        if pair == 0 and k < len(pou_steps):
                            pou_steps[k]()
                        sub_step(ca, sa, k)
                        sub_step(cb, sb, k)

                # ---------- windowed combine ----------
                ez = pst["ez"]
                nc.vector.tensor_mul(u_asm[:], u_asm[:], ez[:])
                pacc = pp.tile([1, PTILE], f32, tag="mm")
                for n in range(NCH):
                    nc.tensor.matmul(
                        pacc[:, n * CHK : (n + 1) * CHK],
                        s_cw[:, 0:1],
                        u_asm[:, n * CHK : (n + 1) * CHK],
                        start=True,
                        stop=True,
                    )
                accrow = rp.tile([2, PTILE], f32, tag="row")
                nc.vector.tensor_copy(accrow[0:1, :], pacc[:])
                nc.sync.dma_start(
                    s_accP[t * 32 : (t + 1) * 32, :], accrow[0:1, :]
                )

            # ---------- final: combine in points-layout ----------
            s_r = fin.tile([128, PPB], f32)
            nc.vector.reciprocal(s_r[:], s_s1P[:])
            s_num = fin.tile([128, PPB], f32)
            nc.vector.tensor_add(s_num[:], s_accP[:], s_s2P[:])
            nc.vector.tensor_mul(s_num[:], s_num[:], s_r[:])
            nc.vector.tensor_mul(s_num[:], s_num[:], s_bc[:])
            nc.sync.dma_start(y.rearrange("(p j) -> p j", p=128), s_num[:])

    nc.compile()
    return nc


def _prep_inputs(inputs):
    f = lambda k: np.ascontiguousarray(np.asarray(inputs[k]), dtype=np.float32)
    x = f("x")
    centers, scales = f("centers"), f("scales")
    sub_W0, sub_b0 = f("sub_W0"), f("sub_b0")
    sub_Wmid, sub_bmid = f("sub_Wmid"), f("sub_bmid")
    sub_Wl, sub_bl = f("sub_Wl"), f("sub_bl")

    # fold per-subdomain normalization into layer-0 weights:
    # xn = (x - c)/s  =>  xn @ W0 + b0 = x @ (W0/s) + (b0 - (c/s) @ W0)
    w0e_full = sub_W0 / scales[:, :, None]                       # [C, D, SW]
    b0e_full = sub_b0 - np.einsum("cd,cdw->cw", centers / scales, sub_W0)

    # row-group packed layer-0 weights: rows {32r,32r+1,32r+2} = [W0; W1; b]
    w0q = np.zeros((128, C * FC * 128), np.float32)
    for c in range(C):
        for fc in range(FC):
            col = (c * FC + fc) * 128
            blk = np.vstack(
                [
                    w0e_full[c][:, fc * 128 : (fc + 1) * 128],
                    b0e_full[c][None, fc * 128 : (fc + 1) * 128],
                ]
            )
            for rr in range(4):
                w0q[32 * rr : 32 * rr + 3, col : col + 128] = blk

    import ml_dtypes

    middt_np = ml_dtypes.bfloat16 if MID_BF16 else np.float32
    wm = np.ascontiguousarray(
        sub_Wmid.reshape(C, SNMID, KC, 128, FC, 128)
        .transpose(3, 0, 1, 4, 2, 5)
        .reshape(128, C * WBLK)
    ).astype(middt_np)
    bm = np.ascontiguousarray(
        sub_bmid.reshape(C, SNMID, FC, 128).transpose(3, 0, 1, 2).reshape(128, -1)
    )
    wl = np.ascontiguousarray(
        sub_Wl.reshape(C, KC, 128).transpose(2, 0, 1).reshape(128, -1)
    ).astype(middt_np)
    cwm = np.ascontiguousarray(
        np.stack([np.ones(C, np.float32), sub_bl[:, 0]], axis=1)
    )

    pw0q = np.zeros((128, PH), np.float32)
    pblk = np.vstack([f("pou_W0"), f("pou_b0")[None, :]])
    for rr in range(4):
        pw0q[32 * rr : 32 * rr + 3, :] = pblk

    shared = dict(
        w0q=w0q,
        wm=wm,
        bm=bm,
        wl=wl,
        pw0q=pw0q,
        pwm=np.ascontiguousarray(f("pou_Wmid").transpose(1, 0, 2).reshape(PH, -1)),
        pbm=np.ascontiguousarray(f("pou_bmid").T),
        pwl=f("pou_Wl"),
        pbl=np.ascontiguousarray(f("pou_bl")[:, None]),
        cw=cwm,
    )

    in_maps = []
    for core in range(NCORES):
        xs = x[core * NP : (core + 1) * NP]
        m = dict(shared)
        m["xT"] = np.ascontiguousarray(
            np.vstack([xs.T, np.ones((1, NP), np.float32)])
        )
        m["xP"] = np.ascontiguousarray(xs.reshape(128, 2 * PPB))
        in_maps.append(m)
    return in_maps


def kernel(**inputs):
    from concourse.bass_utils import run_bass_kernel_spmd

    if "nc" not in _CACHE:
        _enable_ldw_opt()
        _CACHE["nc"] = _build()
    nc = _CACHE["nc"]

    in_maps = _prep_inputs(inputs)
    trace = os.environ.get("KERNEL_TRACE", "0") == "1"
    res = run_bass_kernel_spmd(
        nc, in_maps, core_ids=list(range(NCORES)), trace=trace
    )
    kernel.last_results = res
    y = np.concatenate([res.results[i]["y"] for i in range(NCORES)])
    return y.astype(np.float32)
